# revision 1
# baseline (speedup 1.0000x reference)
"""Bahdanau additive attention kernel for 8 Trainium2 NeuronCores.

Data-parallel over batch: B=64 -> 8 batches per core. No collectives.

Per-batch math (reference):
  Wa   = dec @ Wa_w.T + Wa_b                       [1, H]
  Ua   = enc @ Ua_w.T + Ua_b                       [Te, H]
  s    = tanh(Ua + Wa) @ Va_w.T  (+ Va_b, dropped: softmax shift-invariant)
  w    = softmax(s)                                 [Te]
  ctx  = w @ enc                                    [1, De]

Device layout (per core, 8 batches):
  big matmul Ua:  out[t, h] = sum_d encT[d, t] * uawT[d, h]    (bf16, fp32 PSUM)
  bias:           VectorE add of WaPB broadcast (gpsimd partition_broadcast per b)
  tanh:           ScalarE, SBUF -> SBUF bf16
  score:          VectorE mult by Va bcast (bf16 2x mode) + reduce over h
                  -> scores as columns [128t, 8 chunks]
  softmax:        exp on ScalarE (scores bounded, no max subtraction),
                  normalization deferred to the end (unnormalized weights)
  context:        matmul, exp-weight column stationary vs encN [t, d] moving;
                  software-pipelined one batch behind the score stage and
                  col-tiled: 4 concurrent M=1 chains in PE col-groups
                  0/32/64/96 (tile_position), one 256-wide d-slice + own
                  PSUM bank each (~4x real ctx speedup; the cost model
                  prices matmuls serially and cannot see it)
  scale:          ScalarE copy with scale = 1/sum(exp), per col-group

Cost-model timeline: 282.8us/core (TensorE busy ~250us, within 2% of the
bf16 roofline for the 17.2 GFLOP/core; 45us startup-DMA + drain overhead;
real ctx time is ~20us lower than modeled due to col-group concurrency).
Measured HW rel err 2.7e-3. Non-default options, all measured:
  KERNEL_UA_FP8=1  fp8e4m3+DoubleRow Ua matmul: 155us sim, HW-validated,
                   rel err 1.43e-2 (worst row 1.84e-2) - too close to a
                   2e-2 gate to ship by default.
  ctx_on="vector"  VectorE context reduction over resident encT: 1.9x
                   WORSE (partition-broadcast cost + serial chain).
Rejected by measurement: DMA splitting, prologue reordering/hoisting,
bias on TensorE, buffer-count increases, deferred encN prefetch.
"""

import os
import sys

import numpy as np
import ml_dtypes

for _p in ("/opt/trn_rl_repo",):
    if _p not in sys.path and os.path.isdir(_p):
        sys.path.append(_p)

import concourse.bass as bass
import concourse.tile as tile
import concourse.mybir as mybir
from concourse import bacc
from concourse.bass import ts
from concourse.bass_utils import run_bass_kernel_spmd
from concourse.masks import make_identity

B, T, D, H = 64, 1024, 1024, 1024
NCORES = 8
BPC = B // NCORES  # batches per core
P = 128
DC = D // P  # 8 contraction chunks
TC = T // P  # 8 t chunks

BF = mybir.dt.bfloat16
F8 = mybir.dt.float8e4
F32 = mybir.dt.float32
AF = mybir.ActivationFunctionType
ALU = mybir.AluOpType

# fp8e4m3 + DoubleRow for the Ua matmul (~1.5x TensorE); rel err ~1.4e-2 vs
# bf16's 2.7e-3 (gate 2e-2). Off unless KERNEL_UA_FP8=1.
UA_FP8 = bool(int(os.environ.get("KERNEL_UA_FP8", "0")))
# context matmul on "tensor" (TensorE, needs encN input) or "vector"
# (VectorE reduction over resident encT; drops the encN input entirely)
CTX_ON = os.environ.get("KERNEL_CTX", "tensor")
# run the two context d-halves concurrently in PE col-groups 0/64
CTX_COL2 = bool(int(os.environ.get("KERNEL_CTX_COL2", "1")))
# 4 = four concurrent col-groups (256-wide slices); 0 = use CTX_COL2 setting
CTX_GROUPS = int(os.environ.get("KERNEL_CTX_GROUPS", "4"))


def build_bass(
    bias_on: str = "vector",
    score_bf16: bool = True,
    pipelined: bool = True,
    enc_bufs: int = 2,
    work_bufs: int = 3,
    pu_bufs: int = 4,
    pc_bufs: int = 2,
    wb_via: str = "gpsimd",
    reduce_on: str = "vector",
    dma_split: int = 1,
    n_batches: int = BPC,
    ua_fp8: bool = UA_FP8,
    wapbrow_dma_on: str = "sync",
    hoist_first_enc: bool = False,
    ctx_on: str = "tensor",
    defer_nb0: bool = False,
    ctx_col2: bool = CTX_COL2,
    ctx_groups: int = CTX_GROUPS,
    pc_bufs_override: int | None = None,
):
    if ctx_groups == 4:
        pc_bufs = pc_bufs_override or 4
    nc = bacc.Bacc("TRN2", target_bir_lowering=False, debug=False)

    va_dt = BF if score_bf16 else F32
    th_dt = BF if score_bf16 else F32
    enc_dt = F8 if ua_fp8 else BF
    assert not (ua_fp8 and ctx_on == "vector"), (
        "vector ctx reads EB; fp8 EB is too imprecise for the context reduction"
    )
    if ua_fp8:
        # DoubleRow psum group ends on the K=1 bias matmul; DVE-add path
        # would leave the group open across mixed perf modes.
        bias_on = "tensor"

    encT = nc.dram_tensor("encT", [BPC, D, T], enc_dt, kind="ExternalInput")
    encN = (
        nc.dram_tensor("encN", [BPC, T, D], BF, kind="ExternalInput")
        if ctx_on == "tensor"
        else None
    )
    uawT = nc.dram_tensor("uawT", [D, H], enc_dt, kind="ExternalInput")
    wawT = nc.dram_tensor("wawT", [D, H], BF, kind="ExternalInput")
    decT = nc.dram_tensor("decT", [D, BPC], BF, kind="ExternalInput")
    bsum = nc.dram_tensor("bsum", [1, H], BF, kind="ExternalInput")
    vabc = nc.dram_tensor("vabc", [P, H], va_dt, kind="ExternalInput")
    out = nc.dram_tensor("out", [BPC, D], F32, kind="ExternalOutput")

    with tile.TileContext(nc) as tc:
        with (
            tc.tile_pool(name="const", bufs=1) as cpool,
            tc.tile_pool(name="enc", bufs=enc_bufs) as epool,
            tc.tile_pool(name="work", bufs=work_bufs) as wpool,
            tc.tile_pool(name="pu", bufs=pu_bufs, space="PSUM") as pupool,
            tc.tile_pool(name="pc", bufs=pc_bufs, space="PSUM") as pcpool,
        ):
            def enc_dma(b, skip_nb_dma=False):
                EB = epool.tile([P, DC, T], enc_dt, tag="EB")
                srcT = encT.ap()[b].rearrange("(dc p) t -> p dc t", p=P)
                if ctx_on == "tensor":
                    NB = epool.tile([P, TC, D], BF, tag="NB")
                    srcN = encN.ap()[b].rearrange("(tc p) d -> p tc d", p=P)
                else:
                    NB = None
                split = dma_split if b == 0 else 1
                step = DC // split
                for s in range(split):
                    sl = slice(s * step, (s + 1) * step)
                    nc.sync.dma_start(EB[:, sl, :], srcT[:, sl, :])
                    if NB is not None and not skip_nb_dma:
                        nc.sync.dma_start(NB[:, sl, :], srcN[:, sl, :])
                return EB, NB

            def nb_dma(b, NB):
                srcN = encN.ap()[b].rearrange("(tc p) d -> p tc d", p=P)
                nc.sync.dma_start(NB[:], srcN)

            # batch-0 encoder tiles first: no deps, so the sync queue issues
            # them immediately and they overlap the weight DMAs
            enc0 = enc_dma(0) if hoist_first_enc else None

            # resident weights / constants
            UW = cpool.tile([P, DC, H], enc_dt, tag="UW")
            uw_src = uawT.ap().rearrange("(dc p) h -> p dc h", p=P)
            if dma_split > 1:
                for dc in range(DC):
                    nc.sync.dma_start(UW[:, dc : dc + 1, :], uw_src[:, dc : dc + 1, :])
            else:
                nc.sync.dma_start(UW[:], uw_src)
            WW = cpool.tile([P, DC, H], BF, tag="WW")
            nc.sync.dma_start(WW[:], wawT.ap().rearrange("(dc p) h -> p dc h", p=P))
            DT = cpool.tile([P, DC, BPC], BF, tag="DT")
            nc.sync.dma_start(DT[:], decT.ap().rearrange("(dc p) b -> p dc b", p=P))
            BS = cpool.tile([1, H], BF, tag="BS")
            nc.sync.dma_start(BS[:], bsum.ap())
            VAB = cpool.tile([P, H], va_dt, tag="VAB")
            nc.sync.dma_start(VAB[:], vabc.ap())

            ones_r = cpool.tile([1, P], BF, tag="ones_r")
            nc.vector.memset(ones_r[:], 1.0)
            ones_c = cpool.tile([P, 1], BF, tag="ones_c")
            nc.vector.memset(ones_c[:], 1.0)
            if ctx_on == "vector":
                IDN = cpool.tile([P, P], F32, tag="IDN")
                make_identity(nc, IDN[:])

            # WaPB[b, h] = dec_b @ Wa_w.T + (Wa_b + Ua_b), all batches at once,
            # then flattened to one partition so per-b rows are base-0 matmul rhs.
            WaPBs = cpool.tile([BPC, H], BF, tag="WaPBs")
            for hh in range(2):
                pw = pcpool.tile([BPC, 512], F32, tag="pc")
                for dc in range(DC):
                    nc.tensor.matmul(
                        pw[:],
                        DT[:, dc, :],
                        WW[:, dc, ts(hh, 512)],
                        start=(dc == 0),
                        stop=False,
                    )
                nc.tensor.matmul(
                    pw[:],
                    ones_r[:, 0:BPC],
                    BS[:, ts(hh, 512)],
                    start=False,
                    stop=True,
                )
                nc.vector.tensor_copy(WaPBs[:, ts(hh, 512)], pw[:])
            WaPBrow = cpool.tile([1, BPC * H], BF, tag="WaPBrow")
            # issue these row-flatten DMAs off the sync queue: they carry
            # semaphore waits on the WaPB copies and would head-of-line block
            # the encoder-tile DMAs queued behind them on sync
            wapb_dma = (
                nc.gpsimd.dma_start if wapbrow_dma_on == "gpsimd" else nc.sync.dma_start
            )
            for b in range(BPC):
                wapb_dma(WaPBrow[:, b * H : (b + 1) * H], WaPBs[b : b + 1, :])

            def scores_stage(b, pre=None):
                defer = defer_nb0 and b == 0
                EB, NB = pre if pre is not None else enc_dma(b, skip_nb_dma=defer)

                WaPB = WaPBrow[:, b * H : (b + 1) * H]
                if bias_on == "vector":
                    # broadcast WaPB to 128 partitions once per b
                    if wb_via == "gpsimd":
                        WB = wpool.tile([P, H], BF, tag="WB")
                        nc.gpsimd.partition_broadcast(WB[:], WaPB)
                    else:
                        WB = wpool.tile([P, H], F32, tag="WB")
                        for hh in range(2):
                            pb = pcpool.tile([P, 512], F32, tag="pb")
                            nc.tensor.matmul(
                                pb[:],
                                ones_r[:],
                                WaPB[:, ts(hh, 512)],
                                start=True,
                                stop=True,
                            )
                            nc.vector.tensor_copy(WB[:, ts(hh, 512)], pb[:])
                SC = wpool.tile([P, TC], F32, tag="SC")
                for tci in range(TC):
                    pu0 = pupool.tile([P, 512], F32, tag="pu")
                    pu1 = pupool.tile([P, 512], F32, tag="pu")
                    last = bias_on != "tensor"
                    if ua_fp8:
                        # DoubleRow: contract two 128-chunks per matmul via
                        # 3D APs [128, 2, M] / [128, 2, N]
                        for dc in range(0, DC, 2):
                            lh = EB[:, dc : dc + 2, ts(tci, P)]
                            nc.tensor.matmul(
                                pu0[:],
                                lh,
                                UW[:, dc : dc + 2, 0:512],
                                start=(dc == 0),
                                stop=False,
                                perf_mode=mybir.MatmulPerfMode.DoubleRow,
                            )
                            nc.tensor.matmul(
                                pu1[:],
                                lh,
                                UW[:, dc : dc + 2, 512:1024],
                                start=(dc == 0),
                                stop=False,
                                perf_mode=mybir.MatmulPerfMode.DoubleRow,
                            )
                    else:
                        for dc in range(DC):
                            lh = EB[:, dc, ts(tci, P)]
                            nc.tensor.matmul(
                                pu0[:],
                                lh,
                                UW[:, dc, 0:512],
                                start=(dc == 0),
                                stop=(last and dc == DC - 1),
                            )
                            nc.tensor.matmul(
                                pu1[:],
                                lh,
                                UW[:, dc, 512:1024],
                                start=(dc == 0),
                                stop=(last and dc == DC - 1),
                            )
                    TH = wpool.tile([P, H], th_dt, tag="TH")
                    if bias_on == "tensor":
                        # += WaPB broadcast along t partitions (K=1 ones matmul)
                        nc.tensor.matmul(
                            pu0[:], ones_r[:], WaPB[:, 0:512], start=False, stop=True
                        )
                        nc.tensor.matmul(
                            pu1[:], ones_r[:], WaPB[:, 512:1024], start=False, stop=True
                        )
                        nc.scalar.activation(TH[:, 0:512], pu0[:], AF.Tanh)
                        nc.scalar.activation(TH[:, 512:1024], pu1[:], AF.Tanh)
                    else:
                        T1 = wpool.tile([P, H], F32, tag="T1")
                        nc.vector.tensor_tensor(
                            T1[:, 0:512], pu0[:], WB[:, 0:512], ALU.add
                        )
                        nc.vector.tensor_tensor(
                            T1[:, 512:1024], pu1[:], WB[:, 512:1024], ALU.add
                        )
                        nc.scalar.activation(TH[:, 0:512], T1[:, 0:512], AF.Tanh)
                        nc.scalar.activation(TH[:, 512:1024], T1[:, 512:1024], AF.Tanh)
                    TMP = wpool.tile([P, H], th_dt, tag="TMP")
                    nc.vector.tensor_tensor(TMP[:], TH[:], VAB[:], ALU.mult)
                    if reduce_on == "scalar":
                        TJ = wpool.tile([P, H], th_dt, tag="TJ")
                        nc.scalar.activation(
                            TJ[:],
                            TMP[:],
                            AF.Identity,
                            accum_out=SC[:, tci : tci + 1],
                        )
                    else:
                        nc.vector.tensor_reduce(
                            SC[:, tci : tci + 1],
                            TMP[:],
                            axis=mybir.AxisListType.X,
                            op=ALU.add,
                        )
                if defer and NB is not None:
                    nb_dma(b, NB)
                return SC, NB, EB

            def ctx_stage(b, SC, NB, EB):
                if ctx_on == "vector":
                    return ctx_stage_vector(b, SC, EB)
                # unnormalized softmax weights, bf16 columns [128t, TC]
                EW = wpool.tile([P, TC], BF, tag="EW")
                nc.scalar.activation(EW[:], SC[:], AF.Exp)
                psum_s = pcpool.tile([1, TC], F32, tag="pc")
                nc.tensor.matmul(psum_s[:], ones_c[:], EW[:], start=True, stop=True)
                TOT = wpool.tile([1, 1], F32, tag="TOT")
                nc.vector.tensor_reduce(
                    TOT[:], psum_s[:], axis=mybir.AxisListType.X, op=ALU.add
                )
                INV = wpool.tile([1, 1], F32, tag="INV")
                nc.vector.reciprocal(INV[:], TOT[:])

                if ctx_groups == 4:
                    # four concurrent PE col-groups, one 256-wide d-slice each
                    INV128 = wpool.tile([P, 1], F32, tag="INV128")
                    nc.gpsimd.partition_broadcast(INV128[:], INV[:])
                    bases = (0, 32, 64, 96)
                    pts4 = [
                        pcpool.tile([P, 256], F32, tag="pc", name=f"p4_{b}_{g}")
                        for g in range(4)
                    ]
                    for tci in range(TC):
                        for gi, j in enumerate(bases):
                            nc.tensor.matmul(
                                pts4[gi][j : j + 1, :],
                                EW[:, tci : tci + 1],
                                NB[:, tci, gi * 256 : (gi + 1) * 256],
                                start=(tci == 0),
                                stop=(tci == TC - 1),
                                tile_position=(0, j),
                            )
                    OUTx = wpool.tile([P, 256], F32, tag="OUTx")
                    for gi, j in enumerate(bases):
                        nc.scalar.activation(
                            OUTx[j : j + 1, :],
                            pts4[gi][j : j + 1, :],
                            AF.Copy,
                            scale=INV128[j : j + 1],
                        )
                        nc.sync.dma_start(
                            out.ap()[b : b + 1, gi * 256 : (gi + 1) * 256],
                            OUTx[j : j + 1, :],
                        )
                elif ctx_col2:
                    # run the two d-halves concurrently in PE col-groups 0 and
                    # 64 (tile_position): M=1 uses 1/128 of the array, so the
                    # two matmul chains overlap on HW (~2x ctx speedup; the
                    # cost model prices them serially). One shared PSUM bank,
                    # rows 0 and 64; only the first matmul may carry
                    # start=True — it clears has_written for the whole bank.
                    INV128 = wpool.tile([P, 1], F32, tag="INV128")
                    nc.gpsimd.partition_broadcast(INV128[:], INV[:])
                    pts = [
                        pcpool.tile([P, 512], F32, tag="pc", name=f"pt{b}_0"),
                        pcpool.tile([P, 512], F32, tag="pc", name=f"pt{b}_1"),
                    ]
                    for tci in range(TC):
                        for j, dh in ((0, 0), (64, 1)):
                            nc.tensor.matmul(
                                pts[dh][j : j + 1, :],
                                EW[:, tci : tci + 1],
                                NB[:, tci, ts(dh, 512)],
                                start=(tci == 0),
                                stop=(tci == TC - 1),
                                tile_position=(0, j),
                            )
                    OUTx = wpool.tile([P, 512], F32, tag="OUTx")
                    for j, dh in ((0, 0), (64, 1)):
                        nc.scalar.activation(
                            OUTx[j : j + 1, :],
                            pts[dh][j : j + 1, :],
                            AF.Copy,
                            scale=INV128[j : j + 1],
                        )
                        nc.sync.dma_start(
                            out.ap()[b : b + 1, ts(dh, 512)], OUTx[j : j + 1, :]
                        )
                else:
                    OUTb = wpool.tile([1, D], F32, tag="OUTb")
                    for dh in range(2):
                        pc = pcpool.tile([1, 512], F32, tag="pc")
                        for tci in range(TC):
                            nc.tensor.matmul(
                                pc[:],
                                EW[:, tci : tci + 1],
                                NB[:, tci, ts(dh, 512)],
                                start=(tci == 0),
                                stop=(tci == TC - 1),
                            )
                        nc.scalar.activation(
                            OUTb[:, ts(dh, 512)], pc[:], AF.Copy, scale=INV[:]
                        )
                    nc.sync.dma_start(out.ap()[b : b + 1, :], OUTb[:])

            def ctx_stage_vector(b, SC, EB):
                # scores columns [128t', TC] -> one row [1, T] via PE transpose
                # + flatten DMAs, so exp/softmax-sum run on a single ACT op and
                # the weights can be partition-broadcast for the VectorE
                # context reduction over the already-resident encT tiles.
                pt = pcpool.tile([TC, P], F32, tag="pc")
                nc.tensor.transpose(pt[:], SC[:], IDN[:])
                SROW8 = wpool.tile([TC, P], F32, tag="SROW8")
                nc.vector.tensor_copy(SROW8[:], pt[:])
                SROWf = wpool.tile([1, T], F32, tag="SROWf")
                for tci in range(TC):
                    nc.sync.dma_start(
                        SROWf[:, ts(tci, P)], SROW8[tci : tci + 1, :]
                    )
                EWrow = wpool.tile([1, T], BF, tag="EWrow")
                TOT = wpool.tile([1, 1], F32, tag="TOT")
                nc.scalar.activation(EWrow[:], SROWf[:], AF.Exp, accum_out=TOT[:])
                INV = wpool.tile([1, 1], F32, tag="INV")
                nc.vector.reciprocal(INV[:], TOT[:])
                INV128 = wpool.tile([P, 1], F32, tag="INV128")
                nc.gpsimd.partition_broadcast(INV128[:], INV[:])
                EWbc = wpool.tile([P, T], BF, tag="EWbc")
                nc.gpsimd.partition_broadcast(EWbc[:], EWrow[:])

                CTXc = wpool.tile([P, DC], F32, tag="CTXc")
                for dc in range(DC):
                    TMP2 = wpool.tile([P, T], BF, tag="TMP")
                    nc.vector.tensor_tensor(TMP2[:], EB[:, dc, :], EWbc[:], ALU.mult)
                    nc.vector.tensor_reduce(
                        CTXc[:, dc : dc + 1],
                        TMP2[:],
                        axis=mybir.AxisListType.X,
                        op=ALU.add,
                    )
                nc.vector.tensor_scalar_mul(CTXc[:], CTXc[:], INV128[:])
                nc.sync.dma_start(
                    out.ap()[b].rearrange("(dc p) -> p dc", p=P), CTXc[:]
                )

            if pipelined:
                prev = None
                for b in range(n_batches):
                    cur = scores_stage(b, pre=enc0 if b == 0 else None)
                    if prev is not None:
                        ctx_stage(b - 1, *prev)
                    prev = cur
                ctx_stage(n_batches - 1, *prev)
            else:
                for b in range(n_batches):
                    SC, NB = scores_stage(b, pre=enc0 if b == 0 else None)
                    ctx_stage(b, SC, NB)

    nc.finalize()
    return nc


_NC = None


def _get_nc():
    global _NC
    if _NC is None:
        _NC = build_bass(ctx_on=CTX_ON)
    return _NC


LAST_RESULTS = None


def prepare_in_maps(inputs, ua_fp8: bool = UA_FP8, ctx_on: str = CTX_ON) -> list:
    enc = np.asarray(inputs["encoder_outputs"], dtype=np.float32)  # [B, T, D]
    dec = np.asarray(inputs["decoder_outputs"], dtype=np.float32)[:, 0, :]  # [B, D]
    Wa_w = np.asarray(inputs["Wa_w"], dtype=np.float32)
    Wa_b = np.asarray(inputs["Wa_b"], dtype=np.float32)
    Ua_w = np.asarray(inputs["Ua_w"], dtype=np.float32)
    Ua_b = np.asarray(inputs["Ua_b"], dtype=np.float32)
    Va_w = np.asarray(inputs["Va_w"], dtype=np.float32)
    # Va_b dropped: softmax(s + c) == softmax(s)

    bf16 = ml_dtypes.bfloat16
    enc_t_dt = ml_dtypes.float8_e4m3 if ua_fp8 else bf16
    enc_bf = enc.astype(bf16)  # [B, T, D]
    encN_all = enc_bf.reshape(NCORES, BPC, T, D)
    encT_all = (
        np.ascontiguousarray(enc.transpose(0, 2, 1))
        .astype(enc_t_dt)
        .reshape(NCORES, BPC, D, T)
    )
    decT_all = np.ascontiguousarray(
        dec.reshape(NCORES, BPC, D).transpose(0, 2, 1)
    ).astype(bf16)  # [NCORES, D, BPC]
    uawT = np.ascontiguousarray(Ua_w.T).astype(enc_t_dt)
    wawT = np.ascontiguousarray(Wa_w.T).astype(bf16)
    bsum = (Wa_b + Ua_b).reshape(1, H).astype(bf16)
    vabc = np.ascontiguousarray(np.broadcast_to(Va_w.reshape(1, H), (P, H))).astype(
        bf16
    )

    maps = [
        {
            "encT": np.ascontiguousarray(encT_all[c]),
            "uawT": uawT,
            "wawT": wawT,
            "decT": np.ascontiguousarray(decT_all[c]),
            "bsum": bsum,
            "vabc": vabc,
        }
        for c in range(NCORES)
    ]
    if ctx_on == "tensor":
        for c in range(NCORES):
            maps[c]["encN"] = np.ascontiguousarray(encN_all[c])
    return maps


def kernel(**inputs) -> np.ndarray:
    in_maps = prepare_in_maps(inputs)
    nc = _get_nc()
    trace = bool(int(os.environ.get("KERNEL_TRACE", "0")))
    try:
        res = run_bass_kernel_spmd(
            nc, in_maps, core_ids=list(range(NCORES)), trace=trace
        )
    except ModuleNotFoundError:
        # axon clients without the NTFF hook (antenv.axon_hooks) cannot trace;
        # retry untraced rather than failing the whole run
        os.environ["BASS_NEVER_TRACE"] = "1"
        res = run_bass_kernel_spmd(
            nc, in_maps, core_ids=list(range(NCORES)), trace=False
        )
    global LAST_RESULTS
    LAST_RESULTS = res

    outs = [res.results[c]["out"] for c in range(NCORES)]
    full = np.concatenate(outs, axis=0).reshape(B, 1, D).astype(np.float32)
    return full



# revision 21
# speedup vs baseline: 3.2633x; 3.2633x over previous
"""Bahdanau additive attention kernel for 8 Trainium2 NeuronCores.

Data-parallel over batch: B=64 -> 8 batches per core. No collectives.

Per-batch math (reference):
  Wa   = dec @ Wa_w.T + Wa_b                       [1, H]
  Ua   = enc @ Ua_w.T + Ua_b                       [Te, H]
  s    = tanh(Ua + Wa) @ Va_w.T  (+ Va_b, dropped: softmax shift-invariant)
  w    = softmax(s)                                 [Te]
  ctx  = w @ enc                                    [1, De]

Layout: the big Ua matmul runs in the H-on-partitions orientation
(out[h, t]) with the Ua weights stationary and the fp8 encoder tiles
moving (perf_mode=DoubleRow).  That makes the Wa+bias term a
per-partition scalar, so it fuses into the tanh on ScalarE
(out = tanh(psum/1024 + bias)), and Va becomes the moving operand of
K=128 N=1 score-reduction matmuls (scores accumulate directly in score
column layout [128t', tci]).  exp runs on a [128, 8] tile; the context
is again N=1 matmuls with the bf16 enc[t, d] tiles stationary.

fp8 scaling: enc is quantized as enc*16 and the Ua/Wa weights as w*64
(all powers of two, undone exactly by the tanh activation's scale
argument 1/1024).  The scaling moves the small-magnitude weights out of
the fp8e4m3 subnormal range, which is worth ~20% of the quantization
error budget vs naive casting.

Engine budget per batch (cost model): TensorE 64 DoubleRow matmuls at
N=512 ~6.8us, ScalarE 8 tanh ops over [128, 1024] ~8.4us, DMA 1MB fp8
encT + 2MB bf16 encN ~8.7us.  DMA and ScalarE are the pacing resources.
"""

import os
import sys

import numpy as np
import ml_dtypes

for _p in ("/opt/trn_rl_repo",):
    if _p not in sys.path and os.path.isdir(_p):
        sys.path.append(_p)

import concourse.bass as bass
import concourse.tile as tile
import concourse.mybir as mybir
from concourse import bacc
from concourse import bass_isa
from concourse.bass import ts
from concourse.bass_utils import run_bass_kernel_spmd

B, T, D, H = 64, 1024, 1024, 1024
NCORES = 8
BPC = B // NCORES  # batches per core
P = 128
DC = D // P  # 8 contraction chunks
TC = T // P  # 8 t chunks
HT = H // P  # 8 h chunks

ENC_SCALE = 16.0  # fp8 quantization scale for encoder/decoder activations
W_SCALE = 64.0  # fp8 quantization scale for Ua_w / Wa_w
TOT_SCALE = ENC_SCALE * W_SCALE  # combined scale on the matmul PSUM

BF = mybir.dt.bfloat16
F8 = mybir.dt.float8e4
F32 = mybir.dt.float32
AF = mybir.ActivationFunctionType
ALU = mybir.AluOpType
DR = mybir.MatmulPerfMode.DoubleRow


def build_bass(
    eb_bufs: int = 3,
    nb_bufs: int = 3,
    pu_bufs: int = 3,
    th_bufs: int = 2 * HT,
    n_batches: int = BPC,
    mid_ht: int = 2,
    th_psum: int = 0,  # 1: tanh -> bf16 PSUM, DVE copies to SBUF (pu_bufs=2)
):
    if th_psum:
        pu_bufs = 2
    nc = bacc.Bacc("TRN2", target_bir_lowering=False, debug=False)

    encT = nc.dram_tensor("encT", [BPC, D, T], F8, kind="ExternalInput")
    encN = nc.dram_tensor("encN", [BPC, T, D], BF, kind="ExternalInput")
    uawT = nc.dram_tensor("uawT", [D, H], F8, kind="ExternalInput")
    wawT = nc.dram_tensor("wawT", [D, H], F8, kind="ExternalInput")
    decT = nc.dram_tensor("decT", [D, BPC], F8, kind="ExternalInput")
    bsum = nc.dram_tensor("bsum", [1, H], BF, kind="ExternalInput")  # x1024
    vac = nc.dram_tensor("vac", [P, HT], BF, kind="ExternalInput")
    out = nc.dram_tensor("out", [BPC, P, DC], F32, kind="ExternalOutput")

    with tile.TileContext(nc) as tc:
        with (
            tc.tile_pool(name="const", bufs=1) as cpool,
            tc.tile_pool(name="enc", bufs=2) as epool,
            tc.tile_pool(name="work", bufs=2) as wpool,
            tc.tile_pool(name="pu", bufs=pu_bufs, space="PSUM") as pupool,
            tc.tile_pool(name="pc", bufs=2, space="PSUM") as pcpool,
        ):
            # --- resident weights / constants (DMA order matters: the
            # Wa weights come first so the WaPB bias columns are ready
            # when the first tanh fires) ---
            WW = cpool.tile([P, DC, H], F8, tag="WW")
            nc.sync.dma_start(WW[:], wawT.ap().rearrange("(dc p) h -> p dc h", p=P))
            DT = cpool.tile([P, DC, BPC], F8, tag="DT")
            nc.sync.dma_start(DT[:], decT.ap().rearrange("(dc p) b -> p dc b", p=P))
            BS = cpool.tile([1, H], BF, tag="BS")
            nc.sync.dma_start(BS[:], bsum.ap())
            UW = cpool.tile([P, DC, H], F8, tag="UW")

            ONE8 = cpool.tile([1, BPC], BF, tag="ONE8")
            nc.vector.memset(ONE8[:], 1.0)
            ONEC = cpool.tile([P, 1], F32, tag="ONEC")
            nc.vector.memset(ONEC[:], 1.0)

            def enc_dma_t(b):
                EB = epool.tile([P, DC, T], F8, tag="EB", bufs=eb_bufs)
                src = encT.ap()[b].rearrange("(dc p) t -> p dc t", p=P)
                if b == 0:
                    # batch 0 startup: first t-half, then the Ua weights in
                    # pair-chunks (each matmul group g only needs pair g, so
                    # the first group fires after 1/4 of UW), then the second
                    # t-half
                    nc.sync.dma_start(EB[:, :, 0:512], src[:, :, 0:512])
                    uw_src = uawT.ap().rearrange("(dc p) h -> p dc h", p=P)
                    for g in range(DC // 2):
                        nc.sync.dma_start(
                            UW[:, 2 * g : 2 * g + 2, :], uw_src[:, 2 * g : 2 * g + 2, :]
                        )
                    nc.sync.dma_start(EB[:, :, 512:1024], src[:, :, 512:1024])
                else:
                    nc.sync.dma_start(EB[:], src)
                return EB

            # VAC is only needed by the first score stage; keep it off the
            # startup critical path (WW/UW/EB0 gate the first tanh)
            VAC = cpool.tile([P, HT], BF, tag="VAC")

            def enc_dma_n(b):
                NB = epool.tile([P, TC, D], BF, tag="NB", bufs=nb_bufs)
                nc.sync.dma_start(NB[:], encN.ap()[b].rearrange("(tc p) d -> p tc d", p=P))
                return NB

            # --- WaPB[h, b] = (dec*16) @ (Wa_w*64).T + (Wa_b+Ua_b)*1024,
            # in h-column layout, descaled to fp32 once ---
            WaPBc = cpool.tile([P, HT, BPC], F32, tag="WaPBc")
            for ht in range(HT):
                PW = pcpool.tile([P, BPC], F32, tag="sc8", name=f"pw{ht}")
                for dc in range(DC):
                    nc.tensor.matmul(
                        PW[:],
                        WW[:, dc, ts(ht, P)],
                        DT[:, dc, :],
                        start=(dc == 0),
                        stop=False,
                    )
                nc.tensor.matmul(
                    PW[:], BS[:, ts(ht, P)], ONE8[:], start=False, stop=True
                )
                nc.vector.tensor_scalar_mul(WaPBc[:, ht, :], PW[:], 1.0 / TOT_SCALE)

            def ua_stage(b, EB, mid=None):
                """64 DoubleRow matmuls -> 8 fused bias+tanh tiles [128h, 1024t].

                ``mid`` (the previous batch's score/ctx emission) is spliced in
                after a couple of h-tiles so its exp lands early in the
                strict-FIFO Activation queue without gating this batch's first
                tanh."""
                THs = []
                for ht in range(HT):
                    split0 = b == 0 and ht == 0
                    if split0:
                        # separate per-half PSUM tiles: PSUM deps are
                        # whole-tile, so one [P, T] tile would make the first
                        # half-tanh wait for the second half's matmul group
                        PUh = [
                            pupool.tile([P, 512], F32, tag="pu", name=f"pu_{b}_{ht}_{th}")
                            for th in range(2)
                        ]
                    else:
                        PU = pupool.tile([P, T], F32, tag="pu", name=f"pu_{b}_{ht}")
                    for th in range(2):
                        dst = PUh[th][:] if split0 else PU[:, ts(th, 512)]
                        for g in range(DC // 2):
                            nc.tensor.matmul(
                                dst,
                                UW[:, 2 * g : 2 * g + 2, ts(ht, P)],
                                EB[:, 2 * g : 2 * g + 2, ts(th, 512)],
                                start=(g == 0),
                                stop=(g == DC // 2 - 1),
                                perf_mode=DR,
                            )
                    TH = wpool.tile([P, T], BF, tag="TH", bufs=th_bufs, name=f"th_{b}_{ht}")
                    if split0:
                        # batch 0's first tile activates in t-halves so the
                        # ScalarE stream starts before EB0 fully lands
                        for th in range(2):
                            nc.scalar.activation(
                                TH[:, ts(th, 512)],
                                PUh[th][:],
                                AF.Tanh,
                                bias=WaPBc[:, ht, b : b + 1],
                                scale=1.0 / TOT_SCALE,
                            )
                    else:
                        nc.scalar.activation(
                            TH[:],
                            PU[:],
                            AF.Tanh,
                            bias=WaPBc[:, ht, b : b + 1],
                            scale=1.0 / TOT_SCALE,
                        )
                    THs.append(TH)
                    if ht == mid_ht - 1 and mid is not None:
                        mid()
                if mid is not None and mid_ht > HT:
                    mid()
                return THs

            def score_ctx_stage(b, THs, NB):
                # scores in column layout [128t', tci]: one accumulation
                # group, 64 K=128/N=1 matmuls with the Va column moving
                SCp = pcpool.tile([P, TC], F32, tag="sc8", name=f"sc{b}")
                # ht-outer: the matmuls gated by the last tanh tile are the
                # final 8 of the group, so exp fires right after it lands
                for ht in range(HT):
                    for tci in range(TC):
                        nc.tensor.matmul(
                            SCp[:, tci : tci + 1],
                            THs[ht][:, ts(tci, P)],
                            VAC[:, ht : ht + 1],
                            start=(tci == 0 and ht == 0),
                            stop=(tci == TC - 1 and ht == HT - 1),
                        )
                EW = wpool.tile([P, TC], BF, tag="EW")
                nc.scalar.activation(EW[:], SCp[:], AF.Exp)
                ES = wpool.tile([P, 1], F32, tag="ES")
                nc.vector.tensor_reduce(
                    ES[:], EW[:], axis=mybir.AxisListType.X, op=ALU.add
                )
                # ctx reuses the sc8 PSUM ring: SCp dies at the exp, so the
                # ring alternates SC_b / CTX_b with bufs=2.  Column 8 holds
                # sum(exp): written once before the ctx group, never
                # rewritten, so the group's has_written clears don't hurt it.
                CTXp = pcpool.tile([P, 12], F32, tag="sc8", name=f"ctx{b}")
                nc.tensor.matmul(
                    CTXp[0:1, 8:9], ES[:], ONEC[:], start=True, stop=True
                )
                INV = wpool.tile([1, 1], F32, tag="INV")
                nc.vector.reciprocal(INV[:], CTXp[0:1, 8:9])
                INV128 = wpool.tile([P, 1], F32, tag="INV128")
                nc.gpsimd.partition_broadcast(INV128[:], INV[:])
                for dc in range(DC):
                    for tci in range(TC):
                        nc.tensor.matmul(
                            CTXp[:, dc : dc + 1],
                            NB[:, tci, ts(dc, P)],
                            EW[:, tci : tci + 1],
                            start=(dc == 0 and tci == 0),
                            stop=(dc == DC - 1 and tci == TC - 1),
                        )
                OUTt = wpool.tile([P, DC], F32, tag="OUTt")
                nc.vector.tensor_scalar_mul(OUTt[:], CTXp[:, 0:DC], INV128[:])
                # stores ride the software DGE so they never head-of-line
                # block the input loads on the sync queue
                nc.gpsimd.dma_start(out.ap()[b], OUTt[:])

            # --- software pipeline: EB loads run one batch ahead; the
            # score/ctx stage of batch b-1 is emitted BEFORE the Ua stage
            # of b so its exp/ctx work isn't stuck behind b's tanh block
            # in the strict-FIFO Activation queue ---
            EBs = {0: enc_dma_t(0)}
            nc.sync.dma_start(VAC[:], vac.ap())
            prev = None
            for b in range(n_batches):
                if b + 1 < n_batches:
                    EBs[b + 1] = enc_dma_t(b + 1)
                NB = enc_dma_n(b)
                mid = None
                if prev is not None:
                    pb, pTHs, pNB = b - 1, prev[0], prev[1]
                    mid = lambda: score_ctx_stage(pb, pTHs, pNB)
                THs = ua_stage(b, EBs.pop(b), mid=mid)
                prev = (THs, NB)
            score_ctx_stage(n_batches - 1, *prev)

    nc.finalize()
    return nc


_NC = None


def _get_nc():
    global _NC
    if _NC is None:
        _NC = build_bass()
    return _NC


LAST_RESULTS = None


def prepare_in_maps(inputs) -> list:
    enc = np.asarray(inputs["encoder_outputs"], dtype=np.float32)  # [B, T, D]
    dec = np.asarray(inputs["decoder_outputs"], dtype=np.float32)[:, 0, :]  # [B, D]
    Wa_w = np.asarray(inputs["Wa_w"], dtype=np.float32)
    Wa_b = np.asarray(inputs["Wa_b"], dtype=np.float32)
    Ua_w = np.asarray(inputs["Ua_w"], dtype=np.float32)
    Ua_b = np.asarray(inputs["Ua_b"], dtype=np.float32)
    Va_w = np.asarray(inputs["Va_w"], dtype=np.float32)
    # Va_b dropped: softmax(s + c) == softmax(s)

    bf16 = ml_dtypes.bfloat16
    f8 = ml_dtypes.float8_e4m3

    encN_all = enc.astype(bf16).reshape(NCORES, BPC, T, D)
    encT_all = (
        (np.ascontiguousarray(enc.transpose(0, 2, 1)) * ENC_SCALE)
        .astype(f8)
        .reshape(NCORES, BPC, D, T)
    )
    decT_all = (
        np.ascontiguousarray(dec.reshape(NCORES, BPC, D).transpose(0, 2, 1)) * ENC_SCALE
    ).astype(f8)  # [NCORES, D, BPC]
    uawT8 = (np.ascontiguousarray(Ua_w.T) * W_SCALE).astype(f8)
    wawT8 = (np.ascontiguousarray(Wa_w.T) * W_SCALE).astype(f8)
    bsumS = ((Wa_b + Ua_b) * TOT_SCALE).reshape(1, H).astype(bf16)
    vac = np.ascontiguousarray(Va_w.reshape(HT, P).T).astype(bf16)  # [P, HT]

    return [
        {
            "encT": np.ascontiguousarray(encT_all[c]),
            "encN": np.ascontiguousarray(encN_all[c]),
            "uawT": uawT8,
            "wawT": wawT8,
            "decT": np.ascontiguousarray(decT_all[c]),
            "bsum": bsumS,
            "vac": vac,
        }
        for c in range(NCORES)
    ]


def kernel(**inputs) -> np.ndarray:
    in_maps = prepare_in_maps(inputs)
    nc = _get_nc()
    trace = bool(int(os.environ.get("KERNEL_TRACE", "0")))
    try:
        res = run_bass_kernel_spmd(
            nc, in_maps, core_ids=list(range(NCORES)), trace=trace
        )
    except ModuleNotFoundError:
        # axon clients without the NTFF hook (antenv.axon_hooks) cannot trace;
        # retry untraced rather than failing the whole run
        os.environ["BASS_NEVER_TRACE"] = "1"
        res = run_bass_kernel_spmd(
            nc, in_maps, core_ids=list(range(NCORES)), trace=False
        )
    global LAST_RESULTS
    LAST_RESULTS = res

    # out[b, p, dc] holds ctx[b, dc*128 + p]
    outs = [
        res.results[c]["out"].transpose(0, 2, 1).reshape(BPC, D)
        for c in range(NCORES)
    ]
    return np.concatenate(outs, axis=0).reshape(B, 1, D).astype(np.float32)


# revision 28
# speedup vs baseline: 3.2928x; 1.0090x over previous
"""Bahdanau additive attention kernel for 8 Trainium2 NeuronCores.

Data-parallel over batch: B=64 -> 8 batches per core. No collectives.

Per-batch math (reference):
  Wa   = dec @ Wa_w.T + Wa_b                       [1, H]
  Ua   = enc @ Ua_w.T + Ua_b                       [Te, H]
  s    = tanh(Ua + Wa) @ Va_w.T  (+ Va_b, dropped: softmax shift-invariant)
  w    = softmax(s)                                 [Te]
  ctx  = w @ enc                                    [1, De]

Layout: the big Ua matmul runs in the H-on-partitions orientation
(out[h, t]) with the Ua weights stationary and the fp8 encoder tiles
moving (perf_mode=DoubleRow).  That makes the Wa+bias term a
per-partition scalar, so it fuses into the tanh on ScalarE
(out = tanh(psum/1024 + bias)), and Va becomes the moving operand of
K=128 N=1 score-reduction matmuls (scores accumulate directly in score
column layout [128t', tci]).  exp runs on a [128, 8] tile; the context
is again N=1 matmuls with the bf16 enc[t, d] tiles stationary.

fp8 scaling: enc is quantized as enc*16 and the Ua/Wa weights as w*64
(all powers of two, undone exactly by the tanh activation's scale
argument 1/1024).  The scaling moves the small-magnitude weights out of
the fp8e4m3 subnormal range, which is worth ~20% of the quantization
error budget vs naive casting.

Engine budget per batch (cost model): TensorE 64 DoubleRow matmuls at
N=512 ~6.8us, ScalarE 8 tanh ops over [128, 1024] ~8.4us, DMA 1MB fp8
encT + 2MB bf16 encN ~8.7us.  DMA and ScalarE are the pacing resources.
"""

import os
import sys

import numpy as np
import ml_dtypes

for _p in ("/opt/trn_rl_repo",):
    if _p not in sys.path and os.path.isdir(_p):
        sys.path.append(_p)

import concourse.bass as bass
import concourse.tile as tile
import concourse.mybir as mybir
from concourse import bacc
from concourse import bass_isa
from concourse.bass import ts
from concourse.bass_utils import run_bass_kernel_spmd

B, T, D, H = 64, 1024, 1024, 1024
NCORES = 8
BPC = B // NCORES  # batches per core
P = 128
DC = D // P  # 8 contraction chunks
TC = T // P  # 8 t chunks
HT = H // P  # 8 h chunks

ENC_SCALE = 16.0  # fp8 quantization scale for encoder/decoder activations
W_SCALE = 64.0  # fp8 quantization scale for Ua_w / Wa_w
TOT_SCALE = ENC_SCALE * W_SCALE  # combined scale on the matmul PSUM

BF = mybir.dt.bfloat16
F8 = mybir.dt.float8e4
F32 = mybir.dt.float32
AF = mybir.ActivationFunctionType
ALU = mybir.AluOpType
DR = mybir.MatmulPerfMode.DoubleRow


def build_bass(
    eb_bufs: int = 3,
    nb_bufs: int = 3,
    pu_bufs: int = 3,
    th_bufs: int = 2 * HT,
    n_batches: int = BPC,
    mid_ht: int = 2,
    th_psum: int = 0,  # 1: tanh -> bf16 PSUM, DVE copies to SBUF (pu_bufs=2)
):
    if th_psum:
        pu_bufs = 2
    nc = bacc.Bacc("TRN2", target_bir_lowering=False, debug=False)

    HX = H + BPC  # Wa weights with the decoder columns appended: one DMA,
    # avoids decT's 8-byte rows paying the per-descriptor minimum
    encT = nc.dram_tensor("encT", [BPC, D, T], F8, kind="ExternalInput")
    encN = nc.dram_tensor("encN", [BPC, T, D], BF, kind="ExternalInput")
    uawT = nc.dram_tensor("uawT", [D, H], F8, kind="ExternalInput")
    wawX = nc.dram_tensor("wawX", [D, HX], F8, kind="ExternalInput")
    bsum = nc.dram_tensor("bsum", [1, H], BF, kind="ExternalInput")  # x1024
    vac = nc.dram_tensor("vac", [P, HT], BF, kind="ExternalInput")
    out = nc.dram_tensor("out", [BPC, P, DC], F32, kind="ExternalOutput")

    with tile.TileContext(nc) as tc:
        with (
            tc.tile_pool(name="const", bufs=1) as cpool,
            tc.tile_pool(name="enc", bufs=2) as epool,
            tc.tile_pool(name="work", bufs=2) as wpool,
            tc.tile_pool(name="pu", bufs=pu_bufs, space="PSUM") as pupool,
            tc.tile_pool(name="pc", bufs=2, space="PSUM") as pcpool,
        ):
            # --- resident weights / constants (DMA order matters: the
            # Wa weights come first so the WaPB bias columns are ready
            # when the first tanh fires) ---
            WW = cpool.tile([P, DC, HX], F8, tag="WW")
            nc.sync.dma_start(WW[:], wawX.ap().rearrange("(dc p) h -> p dc h", p=P))
            BS = cpool.tile([1, H], BF, tag="BS")
            nc.sync.dma_start(BS[:], bsum.ap())
            UW = cpool.tile([P, DC, H], F8, tag="UW")

            ONE8 = cpool.tile([1, BPC], BF, tag="ONE8")
            nc.vector.memset(ONE8[:], 1.0)
            ONEC = cpool.tile([P, 1], F32, tag="ONEC")
            nc.vector.memset(ONEC[:], 1.0)

            def enc_dma_t(b):
                EB = epool.tile([P, DC, T], F8, tag="EB", bufs=eb_bufs)
                src = encT.ap()[b].rearrange("(dc p) t -> p dc t", p=P)
                if b == 0:
                    # batch 0 startup: first t-half, then the Ua weights in
                    # pair-chunks (each matmul group g only needs pair g, so
                    # the first group fires after 1/4 of UW), then the second
                    # t-half
                    nc.sync.dma_start(EB[:, :, 0:512], src[:, :, 0:512])
                    uw_src = uawT.ap().rearrange("(dc p) h -> p dc h", p=P)
                    for g in range(DC // 2):
                        nc.sync.dma_start(
                            UW[:, 2 * g : 2 * g + 2, :], uw_src[:, 2 * g : 2 * g + 2, :]
                        )
                    nc.sync.dma_start(EB[:, :, 512:1024], src[:, :, 512:1024])
                else:
                    nc.sync.dma_start(EB[:], src)
                return EB

            # VAC is only needed by the first score stage; keep it off the
            # startup critical path (WW/UW/EB0 gate the first tanh)
            VAC = cpool.tile([P, HT], BF, tag="VAC")

            def enc_dma_n(b):
                NB = epool.tile([P, TC, D], BF, tag="NB", bufs=nb_bufs)
                nc.sync.dma_start(NB[:], encN.ap()[b].rearrange("(tc p) d -> p tc d", p=P))
                return NB

            # --- WaPB[h, b] = (dec*16) @ (Wa_w*64).T + (Wa_b+Ua_b)*1024,
            # in h-column layout, descaled to fp32 once ---
            WaPBc = cpool.tile([P, HT, BPC], F32, tag="WaPBc")
            for ht in range(HT):
                PW = pcpool.tile([P, BPC], F32, tag="sc8", name=f"pw{ht}")
                for dc in range(DC):
                    nc.tensor.matmul(
                        PW[:],
                        WW[:, dc, ts(ht, P)],
                        WW[:, dc, H:HX],
                        start=(dc == 0),
                        stop=False,
                    )
                nc.tensor.matmul(
                    PW[:], BS[:, ts(ht, P)], ONE8[:], start=False, stop=True
                )
                nc.vector.tensor_scalar_mul(WaPBc[:, ht, :], PW[:], 1.0 / TOT_SCALE)

            def ua_stage(b, EB, mid=None):
                """64 DoubleRow matmuls -> 8 fused bias+tanh tiles [128h, 1024t].

                ``mid`` (the previous batch's score/ctx emission) is spliced in
                after a couple of h-tiles so its exp lands early in the
                strict-FIFO Activation queue without gating this batch's first
                tanh."""
                THs = []
                for ht in range(HT):
                    split0 = b == 0 and ht == 0
                    if split0:
                        # separate per-half PSUM tiles: PSUM deps are
                        # whole-tile, so one [P, T] tile would make the first
                        # half-tanh wait for the second half's matmul group
                        PUh = [
                            pupool.tile([P, 512], F32, tag="pu", name=f"pu_{b}_{ht}_{th}")
                            for th in range(2)
                        ]
                    else:
                        PU = pupool.tile([P, T], F32, tag="pu", name=f"pu_{b}_{ht}")
                    for th in range(2):
                        dst = PUh[th][:] if split0 else PU[:, ts(th, 512)]
                        for g in range(DC // 2):
                            nc.tensor.matmul(
                                dst,
                                UW[:, 2 * g : 2 * g + 2, ts(ht, P)],
                                EB[:, 2 * g : 2 * g + 2, ts(th, 512)],
                                start=(g == 0),
                                stop=(g == DC // 2 - 1),
                                perf_mode=DR,
                            )
                    TH = wpool.tile([P, T], BF, tag="TH", bufs=th_bufs, name=f"th_{b}_{ht}")
                    if th_psum:
                        # tanh lands in a bf16 PSUM tile (cheaper ScalarE
                        # access) and the idle DVE stages it to SBUF for the
                        # score matmuls
                        THp = pcpool.tile([P, T], BF, tag="thp", bufs=2, name=f"thp_{b}_{ht}")
                        act_out = THp
                    else:
                        act_out = TH
                    if split0:
                        # batch 0's first tile activates in t-halves so the
                        # ScalarE stream starts before EB0 fully lands
                        for th in range(2):
                            nc.scalar.activation(
                                act_out[:, ts(th, 512)],
                                PUh[th][:],
                                AF.Tanh,
                                bias=WaPBc[:, ht, b : b + 1],
                                scale=1.0 / TOT_SCALE,
                            )
                    else:
                        nc.scalar.activation(
                            act_out[:],
                            PU[:],
                            AF.Tanh,
                            bias=WaPBc[:, ht, b : b + 1],
                            scale=1.0 / TOT_SCALE,
                        )
                    if th_psum:
                        nc.vector.tensor_copy(TH[:], THp[:])
                    THs.append(TH)
                    if ht == mid_ht - 1 and mid is not None:
                        mid()
                if mid is not None and mid_ht > HT:
                    mid()
                return THs

            def score_ctx_stage(b, THs, NB):
                # scores in column layout [128t', tci]: one accumulation
                # group, 64 K=128/N=1 matmuls with the Va column moving
                SCp = pcpool.tile([P, TC], F32, tag="sc8", name=f"sc{b}")
                # ht-outer: the matmuls gated by the last tanh tile are the
                # final 8 of the group, so exp fires right after it lands
                for ht in range(HT):
                    for tci in range(TC):
                        nc.tensor.matmul(
                            SCp[:, tci : tci + 1],
                            THs[ht][:, ts(tci, P)],
                            VAC[:, ht : ht + 1],
                            start=(tci == 0 and ht == 0),
                            stop=(tci == TC - 1 and ht == HT - 1),
                        )
                EW = wpool.tile([P, TC], BF, tag="EW")
                nc.scalar.activation(EW[:], SCp[:], AF.Exp)
                ES = wpool.tile([P, 1], F32, tag="ES")
                nc.vector.tensor_reduce(
                    ES[:], EW[:], axis=mybir.AxisListType.X, op=ALU.add
                )
                # ctx reuses the sc8 PSUM ring: SCp dies at the exp, so the
                # ring alternates SC_b / CTX_b with bufs=2.  Column 8 holds
                # sum(exp): written once before the ctx group, never
                # rewritten, so the group's has_written clears don't hurt it.
                CTXp = pcpool.tile([P, 12], F32, tag="sc8", name=f"ctx{b}")
                nc.tensor.matmul(
                    CTXp[0:1, 8:9], ES[:], ONEC[:], start=True, stop=True
                )
                INV = wpool.tile([1, 1], F32, tag="INV")
                nc.vector.reciprocal(INV[:], CTXp[0:1, 8:9])
                INV128 = wpool.tile([P, 1], F32, tag="INV128")
                nc.gpsimd.partition_broadcast(INV128[:], INV[:])
                for dc in range(DC):
                    for tci in range(TC):
                        nc.tensor.matmul(
                            CTXp[:, dc : dc + 1],
                            NB[:, tci, ts(dc, P)],
                            EW[:, tci : tci + 1],
                            start=(dc == 0 and tci == 0),
                            stop=(dc == DC - 1 and tci == TC - 1),
                        )
                OUTt = wpool.tile([P, DC], F32, tag="OUTt")
                nc.vector.tensor_scalar_mul(OUTt[:], CTXp[:, 0:DC], INV128[:])
                # mid-stream stores ride the software DGE so they never
                # head-of-line block the input loads on the sync queue; the
                # last one takes the idle Activation HWDGE (lower latency)
                if b == n_batches - 1:
                    nc.scalar.dma_start(out.ap()[b], OUTt[:])
                else:
                    nc.gpsimd.dma_start(out.ap()[b], OUTt[:])

            # --- software pipeline: EB loads run one batch ahead; the
            # score/ctx stage of batch b-1 is emitted BEFORE the Ua stage
            # of b so its exp/ctx work isn't stuck behind b's tanh block
            # in the strict-FIFO Activation queue ---
            EBs = {0: enc_dma_t(0)}
            nc.sync.dma_start(VAC[:], vac.ap())
            prev = None
            for b in range(n_batches):
                if b + 1 < n_batches:
                    EBs[b + 1] = enc_dma_t(b + 1)
                NB = enc_dma_n(b)
                mid = None
                if prev is not None:
                    pb, pTHs, pNB = b - 1, prev[0], prev[1]
                    mid = lambda: score_ctx_stage(pb, pTHs, pNB)
                THs = ua_stage(b, EBs.pop(b), mid=mid)
                prev = (THs, NB)
            score_ctx_stage(n_batches - 1, *prev)

    nc.finalize()
    return nc


_NC = None


def _get_nc():
    global _NC
    if _NC is None:
        _NC = build_bass()
    return _NC


LAST_RESULTS = None


def prepare_in_maps(inputs) -> list:
    enc = np.asarray(inputs["encoder_outputs"], dtype=np.float32)  # [B, T, D]
    dec = np.asarray(inputs["decoder_outputs"], dtype=np.float32)[:, 0, :]  # [B, D]
    Wa_w = np.asarray(inputs["Wa_w"], dtype=np.float32)
    Wa_b = np.asarray(inputs["Wa_b"], dtype=np.float32)
    Ua_w = np.asarray(inputs["Ua_w"], dtype=np.float32)
    Ua_b = np.asarray(inputs["Ua_b"], dtype=np.float32)
    Va_w = np.asarray(inputs["Va_w"], dtype=np.float32)
    # Va_b dropped: softmax(s + c) == softmax(s)

    bf16 = ml_dtypes.bfloat16
    f8 = ml_dtypes.float8_e4m3

    encN_all = enc.astype(bf16).reshape(NCORES, BPC, T, D)
    encT_all = (
        (np.ascontiguousarray(enc.transpose(0, 2, 1)) * ENC_SCALE)
        .astype(f8)
        .reshape(NCORES, BPC, D, T)
    )
    decT_all = (
        np.ascontiguousarray(dec.reshape(NCORES, BPC, D).transpose(0, 2, 1)) * ENC_SCALE
    ).astype(f8)  # [NCORES, D, BPC]
    uawT8 = (np.ascontiguousarray(Ua_w.T) * W_SCALE).astype(f8)
    wawT8 = (np.ascontiguousarray(Wa_w.T) * W_SCALE).astype(f8)
    wawX_all = [
        np.ascontiguousarray(np.concatenate([wawT8, decT_all[c]], axis=1))
        for c in range(NCORES)
    ]
    bsumS = ((Wa_b + Ua_b) * TOT_SCALE).reshape(1, H).astype(bf16)
    vac = np.ascontiguousarray(Va_w.reshape(HT, P).T).astype(bf16)  # [P, HT]

    return [
        {
            "encT": np.ascontiguousarray(encT_all[c]),
            "encN": np.ascontiguousarray(encN_all[c]),
            "uawT": uawT8,
            "wawX": wawX_all[c],
            "bsum": bsumS,
            "vac": vac,
        }
        for c in range(NCORES)
    ]


def kernel(**inputs) -> np.ndarray:
    in_maps = prepare_in_maps(inputs)
    nc = _get_nc()
    trace = bool(int(os.environ.get("KERNEL_TRACE", "0")))
    try:
        res = run_bass_kernel_spmd(
            nc, in_maps, core_ids=list(range(NCORES)), trace=trace
        )
    except ModuleNotFoundError:
        # axon clients without the NTFF hook (antenv.axon_hooks) cannot trace;
        # retry untraced rather than failing the whole run
        os.environ["BASS_NEVER_TRACE"] = "1"
        res = run_bass_kernel_spmd(
            nc, in_maps, core_ids=list(range(NCORES)), trace=False
        )
    global LAST_RESULTS
    LAST_RESULTS = res

    # out[b, p, dc] holds ctx[b, dc*128 + p]
    outs = [
        res.results[c]["out"].transpose(0, 2, 1).reshape(BPC, D)
        for c in range(NCORES)
    ]
    return np.concatenate(outs, axis=0).reshape(B, 1, D).astype(np.float32)


# revision 34
# speedup vs baseline: 3.3484x; 1.0169x over previous
"""Bahdanau additive attention kernel for 8 Trainium2 NeuronCores.

Data-parallel over batch: B=64 -> 8 batches per core. No collectives.

Per-batch math (reference):
  Wa   = dec @ Wa_w.T + Wa_b                       [1, H]
  Ua   = enc @ Ua_w.T + Ua_b                       [Te, H]
  s    = tanh(Ua + Wa) @ Va_w.T  (+ Va_b, dropped: softmax shift-invariant)
  w    = softmax(s)                                 [Te]
  ctx  = w @ enc                                    [1, De]

Layout: the big Ua matmul runs in the H-on-partitions orientation
(out[h, t]) with the Ua weights stationary and the fp8 encoder tiles
moving (perf_mode=DoubleRow).  That makes the Wa+bias term a
per-partition scalar, so it fuses into the tanh on ScalarE
(out = tanh(psum/1024 + bias)), and Va becomes the moving operand of
K=128 N=1 score-reduction matmuls (scores accumulate directly in score
column layout [128t', tci]).  exp runs on a [128, 8] tile; the context
is again N=1 matmuls with the bf16 enc[t, d] tiles stationary.

fp8 scaling: enc is quantized as enc*16 and the Ua/Wa weights as w*64
(all powers of two, undone exactly by the tanh activation's scale
argument 1/1024).  The scaling moves the small-magnitude weights out of
the fp8e4m3 subnormal range, which is worth ~20% of the quantization
error budget vs naive casting.

Engine budget per batch (cost model): TensorE 64 DoubleRow matmuls at
N=512 ~6.8us, ScalarE 8 tanh ops over [128, 1024] ~8.4us, DMA 1MB fp8
encT + 2MB bf16 encN ~8.7us.  DMA and ScalarE are the pacing resources.
"""

import os
import sys

import numpy as np
import ml_dtypes

for _p in ("/opt/trn_rl_repo",):
    if _p not in sys.path and os.path.isdir(_p):
        sys.path.append(_p)

import concourse.bass as bass
import concourse.tile as tile
import concourse.mybir as mybir
from concourse import bacc
from concourse import bass_isa
from concourse.bass import ts
from concourse.bass_utils import run_bass_kernel_spmd

B, T, D, H = 64, 1024, 1024, 1024
NCORES = 8
BPC = B // NCORES  # batches per core
P = 128
DC = D // P  # 8 contraction chunks
TC = T // P  # 8 t chunks
HT = H // P  # 8 h chunks

ENC_SCALE = 16.0  # fp8 quantization scale for encoder/decoder activations
W_SCALE = 64.0  # fp8 quantization scale for Ua_w / Wa_w
TOT_SCALE = ENC_SCALE * W_SCALE  # combined scale on the matmul PSUM

BF = mybir.dt.bfloat16
F8 = mybir.dt.float8e4
F32 = mybir.dt.float32
AF = mybir.ActivationFunctionType
ALU = mybir.AluOpType
DR = mybir.MatmulPerfMode.DoubleRow


def build_bass(
    eb_bufs: int = 3,
    nb_bufs: int = 3,
    pu_bufs: int = 3,
    th_bufs: int = 2 * HT,
    n_batches: int = BPC,
    mid_ht: int = 2,
    th_psum: int = 0,  # 1: tanh -> bf16 PSUM, DVE copies to SBUF (pu_bufs=2)
):
    if th_psum:
        pu_bufs = 2
    nc = bacc.Bacc("TRN2", target_bir_lowering=False, debug=False)

    HX = H + BPC  # Wa weights with the decoder columns appended: one DMA,
    # avoids decT's 8-byte rows paying the per-descriptor minimum
    encT = nc.dram_tensor("encT", [BPC, D, T], F8, kind="ExternalInput")
    encN = nc.dram_tensor("encN", [BPC, T, D], BF, kind="ExternalInput")
    uawT = nc.dram_tensor("uawT", [D, H], F8, kind="ExternalInput")
    wawX = nc.dram_tensor("wawX", [D, HX], F8, kind="ExternalInput")
    bsum = nc.dram_tensor("bsum", [1, H], BF, kind="ExternalInput")  # x1024
    vac = nc.dram_tensor("vac", [P, HT], BF, kind="ExternalInput")
    out = nc.dram_tensor("out", [BPC, P, DC], F32, kind="ExternalOutput")

    with tile.TileContext(nc) as tc:
        with (
            tc.tile_pool(name="const", bufs=1) as cpool,
            tc.tile_pool(name="enc", bufs=2) as epool,
            tc.tile_pool(name="work", bufs=2) as wpool,
            tc.tile_pool(name="pu", bufs=pu_bufs, space="PSUM") as pupool,
            tc.tile_pool(name="pc", bufs=2, space="PSUM") as pcpool,
        ):
            # --- resident weights / constants (DMA order matters: the
            # Wa weights come first so the WaPB bias columns are ready
            # when the first tanh fires) ---
            WW = cpool.tile([P, DC, HX], F8, tag="WW")
            nc.sync.dma_start(WW[:], wawX.ap().rearrange("(dc p) h -> p dc h", p=P))
            BS = cpool.tile([1, H], BF, tag="BS")
            nc.sync.dma_start(BS[:], bsum.ap())
            UW = cpool.tile([P, DC, H], F8, tag="UW")

            ONE8 = cpool.tile([1, BPC], BF, tag="ONE8")
            nc.vector.memset(ONE8[:], 1.0)
            ONEPP = cpool.tile([P, P], BF, tag="ONEPP")
            nc.vector.memset(ONEPP[:], 1.0)

            def enc_dma_t(b):
                EB = epool.tile([P, DC, T], F8, tag="EB", bufs=eb_bufs)
                src = encT.ap()[b].rearrange("(dc p) t -> p dc t", p=P)
                if b == 0:
                    # batch 0 startup: the Ua weights split by h-column
                    # halves — h-tiles 0-3 only need the first half, so the
                    # tanh stream starts ~1.5us before the full weight load
                    uw_src = uawT.ap().rearrange("(dc p) h -> p dc h", p=P)
                    nc.sync.dma_start(UW[:, :, 0:512], uw_src[:, :, 0:512])
                    nc.sync.dma_start(EB[:, :, 0:512], src[:, :, 0:512])
                    nc.sync.dma_start(EB[:, :, 512:1024], src[:, :, 512:1024])
                    nc.sync.dma_start(UW[:, :, 512:1024], uw_src[:, :, 512:1024])
                else:
                    nc.sync.dma_start(EB[:], src)
                return EB

            # VAC is only needed by the first score stage; keep it off the
            # startup critical path (WW/UW/EB0 gate the first tanh)
            VAC = cpool.tile([P, HT], BF, tag="VAC")

            def enc_dma_n(b):
                NB = epool.tile([P, TC, D], BF, tag="NB", bufs=nb_bufs)
                nc.sync.dma_start(NB[:], encN.ap()[b].rearrange("(tc p) d -> p tc d", p=P))
                return NB

            # --- WaPB[h, b] = (dec*16) @ (Wa_w*64).T + (Wa_b+Ua_b)*1024,
            # in h-column layout, descaled to fp32 once ---
            WaPBc = cpool.tile([P, HT, BPC], F32, tag="WaPBc")
            for ht in range(HT):
                PW = pcpool.tile([P, BPC], F32, tag="sc8", name=f"pw{ht}")
                for dc in range(DC):
                    nc.tensor.matmul(
                        PW[:],
                        WW[:, dc, ts(ht, P)],
                        WW[:, dc, H:HX],
                        start=(dc == 0),
                        stop=False,
                    )
                nc.tensor.matmul(
                    PW[:], BS[:, ts(ht, P)], ONE8[:], start=False, stop=True
                )
                nc.vector.tensor_scalar_mul(WaPBc[:, ht, :], PW[:], 1.0 / TOT_SCALE)

            def ua_stage(b, EB, mid=None):
                """64 DoubleRow matmuls -> 8 fused bias+tanh tiles [128h, 1024t].

                ``mid`` (the previous batch's score/ctx emission) is spliced in
                after a couple of h-tiles so its exp lands early in the
                strict-FIFO Activation queue without gating this batch's first
                tanh."""
                THs = []
                for ht in range(HT):
                    split0 = b == 0 and ht == 0
                    if split0:
                        # separate per-half PSUM tiles: PSUM deps are
                        # whole-tile, so one [P, T] tile would make the first
                        # half-tanh wait for the second half's matmul group
                        PUh = [
                            pupool.tile([P, 512], F32, tag="pu", name=f"pu_{b}_{ht}_{th}")
                            for th in range(2)
                        ]
                    else:
                        PU = pupool.tile([P, T], F32, tag="pu", name=f"pu_{b}_{ht}")
                    for th in range(2):
                        dst = PUh[th][:] if split0 else PU[:, ts(th, 512)]
                        for g in range(DC // 2):
                            nc.tensor.matmul(
                                dst,
                                UW[:, 2 * g : 2 * g + 2, ts(ht, P)],
                                EB[:, 2 * g : 2 * g + 2, ts(th, 512)],
                                start=(g == 0),
                                stop=(g == DC // 2 - 1),
                                perf_mode=DR,
                            )
                    TH = wpool.tile([P, T], BF, tag="TH", bufs=th_bufs, name=f"th_{b}_{ht}")
                    if th_psum:
                        # tanh lands in a bf16 PSUM tile (cheaper ScalarE
                        # access) and the idle DVE stages it to SBUF for the
                        # score matmuls
                        THp = pcpool.tile([P, T], BF, tag="thp", bufs=2, name=f"thp_{b}_{ht}")
                        act_out = THp
                    else:
                        act_out = TH
                    if split0:
                        # batch 0's first tile activates in t-halves so the
                        # ScalarE stream starts before EB0 fully lands
                        for th in range(2):
                            nc.scalar.activation(
                                act_out[:, ts(th, 512)],
                                PUh[th][:],
                                AF.Tanh,
                                bias=WaPBc[:, ht, b : b + 1],
                                scale=1.0 / TOT_SCALE,
                            )
                    else:
                        nc.scalar.activation(
                            act_out[:],
                            PU[:],
                            AF.Tanh,
                            bias=WaPBc[:, ht, b : b + 1],
                            scale=1.0 / TOT_SCALE,
                        )
                    if th_psum:
                        nc.vector.tensor_copy(TH[:], THp[:])
                    THs.append(TH)
                    if ht == mid_ht - 1 and mid is not None:
                        mid()
                if mid is not None and mid_ht > HT:
                    mid()
                return THs

            def score_ctx_stage(b, THs, NB):
                # scores in column layout [128t', tci]: one accumulation
                # group, 64 K=128/N=1 matmuls with the Va column moving
                SCp = pcpool.tile([P, TC], F32, tag="sc8", name=f"sc{b}")
                # ht-outer: the matmuls gated by the last tanh tile are the
                # final 8 of the group, so exp fires right after it lands
                for ht in range(HT):
                    for tci in range(TC):
                        nc.tensor.matmul(
                            SCp[:, tci : tci + 1],
                            THs[ht][:, ts(tci, P)],
                            VAC[:, ht : ht + 1],
                            start=(tci == 0 and ht == 0),
                            stop=(tci == TC - 1 and ht == HT - 1),
                        )
                EW = wpool.tile([P, TC], BF, tag="EW")
                nc.scalar.activation(EW[:], SCp[:], AF.Exp)
                ES = wpool.tile([P, 1], BF, tag="ES")
                # bf16 partial sums of 8 positive exps: ~0.4% on the softmax
                # denominator, far under the fp8 quantization noise
                with nc.allow_low_precision(reason="softmax denom in bf16"):
                    nc.vector.tensor_reduce(
                        ES[:], EW[:], axis=mybir.AxisListType.X, op=ALU.add
                    )
                # ctx reuses the sc8 PSUM ring: SCp dies at the exp, so the
                # ring alternates SC_b / CTX_b with bufs=2.  Column 8 holds
                # sum(exp) replicated to every partition (all-ones stationary
                # matmul), skipping a gpsimd broadcast hop; its group runs
                # after the ctx group (it only gates the final scale), so the
                # PE never stalls on the DVE reduce before starting ctx.
                CTXp = pcpool.tile([P, 12], F32, tag="sc8", name=f"ctx{b}")
                for dc in range(DC):
                    for tci in range(TC):
                        nc.tensor.matmul(
                            CTXp[:, dc : dc + 1],
                            NB[:, tci, ts(dc, P)],
                            EW[:, tci : tci + 1],
                            start=(dc == 0 and tci == 0),
                            stop=(dc == DC - 1 and tci == TC - 1),
                        )
                nc.tensor.matmul(
                    CTXp[:, 8:9], ONEPP[:], ES[:], start=True, stop=True
                )
                INV128 = wpool.tile([P, 1], F32, tag="INV128")
                nc.vector.reciprocal(INV128[:], CTXp[:, 8:9])
                OUTt = wpool.tile([P, DC], F32, tag="OUTt", bufs=4)
                nc.vector.tensor_scalar_mul(OUTt[:], CTXp[:, 0:DC], INV128[:])
                # mid-stream stores ride the software DGE so they never
                # head-of-line block the input loads on the sync queue; the
                # last one takes the idle Activation HWDGE (lower latency)
                if b == n_batches - 1:
                    nc.scalar.dma_start(out.ap()[b], OUTt[:])
                else:
                    nc.gpsimd.dma_start(out.ap()[b], OUTt[:])

            # --- software pipeline: EB loads run one batch ahead; the
            # score/ctx stage of batch b-1 is emitted BEFORE the Ua stage
            # of b so its exp/ctx work isn't stuck behind b's tanh block
            # in the strict-FIFO Activation queue ---
            EBs = {0: enc_dma_t(0)}
            nc.sync.dma_start(VAC[:], vac.ap())
            prev = None
            for b in range(n_batches):
                if b + 1 < n_batches:
                    EBs[b + 1] = enc_dma_t(b + 1)
                NB = enc_dma_n(b)
                mid = None
                if prev is not None:
                    pb, pTHs, pNB = b - 1, prev[0], prev[1]
                    mid = lambda: score_ctx_stage(pb, pTHs, pNB)
                THs = ua_stage(b, EBs.pop(b), mid=mid)
                prev = (THs, NB)
            score_ctx_stage(n_batches - 1, *prev)

    nc.finalize()
    return nc


_NC = None


def _get_nc():
    global _NC
    if _NC is None:
        _NC = build_bass()
    return _NC


LAST_RESULTS = None


def prepare_in_maps(inputs) -> list:
    enc = np.asarray(inputs["encoder_outputs"], dtype=np.float32)  # [B, T, D]
    dec = np.asarray(inputs["decoder_outputs"], dtype=np.float32)[:, 0, :]  # [B, D]
    Wa_w = np.asarray(inputs["Wa_w"], dtype=np.float32)
    Wa_b = np.asarray(inputs["Wa_b"], dtype=np.float32)
    Ua_w = np.asarray(inputs["Ua_w"], dtype=np.float32)
    Ua_b = np.asarray(inputs["Ua_b"], dtype=np.float32)
    Va_w = np.asarray(inputs["Va_w"], dtype=np.float32)
    # Va_b dropped: softmax(s + c) == softmax(s)

    bf16 = ml_dtypes.bfloat16
    f8 = ml_dtypes.float8_e4m3

    encN_all = enc.astype(bf16).reshape(NCORES, BPC, T, D)
    encT_all = (
        (np.ascontiguousarray(enc.transpose(0, 2, 1)) * ENC_SCALE)
        .astype(f8)
        .reshape(NCORES, BPC, D, T)
    )
    decT_all = (
        np.ascontiguousarray(dec.reshape(NCORES, BPC, D).transpose(0, 2, 1)) * ENC_SCALE
    ).astype(f8)  # [NCORES, D, BPC]
    uawT8 = (np.ascontiguousarray(Ua_w.T) * W_SCALE).astype(f8)
    wawT8 = (np.ascontiguousarray(Wa_w.T) * W_SCALE).astype(f8)
    wawX_all = [
        np.ascontiguousarray(np.concatenate([wawT8, decT_all[c]], axis=1))
        for c in range(NCORES)
    ]
    bsumS = ((Wa_b + Ua_b) * TOT_SCALE).reshape(1, H).astype(bf16)
    vac = np.ascontiguousarray(Va_w.reshape(HT, P).T).astype(bf16)  # [P, HT]

    return [
        {
            "encT": np.ascontiguousarray(encT_all[c]),
            "encN": np.ascontiguousarray(encN_all[c]),
            "uawT": uawT8,
            "wawX": wawX_all[c],
            "bsum": bsumS,
            "vac": vac,
        }
        for c in range(NCORES)
    ]


def kernel(**inputs) -> np.ndarray:
    in_maps = prepare_in_maps(inputs)
    nc = _get_nc()
    trace = bool(int(os.environ.get("KERNEL_TRACE", "0")))
    try:
        res = run_bass_kernel_spmd(
            nc, in_maps, core_ids=list(range(NCORES)), trace=trace
        )
    except ModuleNotFoundError:
        # axon clients without the NTFF hook (antenv.axon_hooks) cannot trace;
        # retry untraced rather than failing the whole run
        os.environ["BASS_NEVER_TRACE"] = "1"
        res = run_bass_kernel_spmd(
            nc, in_maps, core_ids=list(range(NCORES)), trace=False
        )
    global LAST_RESULTS
    LAST_RESULTS = res

    # out[b, p, dc] holds ctx[b, dc*128 + p]
    outs = [
        res.results[c]["out"].transpose(0, 2, 1).reshape(BPC, D)
        for c in range(NCORES)
    ]
    return np.concatenate(outs, axis=0).reshape(B, 1, D).astype(np.float32)


# revision 41
# speedup vs baseline: 3.3982x; 1.0149x over previous
"""Bahdanau additive attention kernel for 8 Trainium2 NeuronCores.

Data-parallel over batch: B=64 -> 8 batches per core. No collectives.

Per-batch math (reference):
  Wa   = dec @ Wa_w.T + Wa_b                       [1, H]
  Ua   = enc @ Ua_w.T + Ua_b                       [Te, H]
  s    = tanh(Ua + Wa) @ Va_w.T  (+ Va_b, dropped: softmax shift-invariant)
  w    = softmax(s)                                 [Te]
  ctx  = w @ enc                                    [1, De]

Layout: the big Ua matmul runs in the H-on-partitions orientation
(out[h, t]) with the Ua weights stationary and the fp8 encoder tiles
moving (perf_mode=DoubleRow).  That makes the Wa+bias term a
per-partition scalar, so it fuses into the tanh on ScalarE
(out = tanh(psum/1024 + bias)), and Va becomes the moving operand of
K=128 N=1 score-reduction matmuls (scores accumulate directly in score
column layout [128t', tci]).  exp runs on a [128, 8] tile; the context
is again N=1 matmuls with the bf16 enc[t, d] tiles stationary.

fp8 scaling: enc is quantized as enc*16 and the Ua/Wa weights as w*64
(all powers of two, undone exactly by the tanh activation's scale
argument 1/1024).  The scaling moves the small-magnitude weights out of
the fp8e4m3 subnormal range, which is worth ~20% of the quantization
error budget vs naive casting.

Engine budget per batch (cost model): TensorE 64 DoubleRow matmuls at
N=512 ~6.8us, ScalarE 8 tanh ops over [128, 1024] ~8.4us, DMA 1MB fp8
encT + 2MB bf16 encN ~8.7us.  DMA and ScalarE are the pacing resources.
"""

import os
import sys

import numpy as np
import ml_dtypes

for _p in ("/opt/trn_rl_repo",):
    if _p not in sys.path and os.path.isdir(_p):
        sys.path.append(_p)

import concourse.bass as bass
import concourse.tile as tile
import concourse.mybir as mybir
from concourse import bacc
from concourse import bass_isa
from concourse.bass import ts
from concourse.bass_utils import run_bass_kernel_spmd

B, T, D, H = 64, 1024, 1024, 1024
NCORES = 8
BPC = B // NCORES  # batches per core
P = 128
DC = D // P  # 8 contraction chunks
TC = T // P  # 8 t chunks
HT = H // P  # 8 h chunks

ENC_SCALE = 16.0  # fp8 quantization scale for encoder/decoder activations
W_SCALE = 64.0  # fp8 quantization scale for Ua_w / Wa_w
TOT_SCALE = ENC_SCALE * W_SCALE  # combined scale on the matmul PSUM

BF = mybir.dt.bfloat16
F8 = mybir.dt.float8e4
F32 = mybir.dt.float32
AF = mybir.ActivationFunctionType
ALU = mybir.AluOpType
DR = mybir.MatmulPerfMode.DoubleRow


def build_bass(
    eb_bufs: int = 3,
    nb_bufs: int = 3,
    pu_bufs: int = 3,
    th_bufs: int = 2 * HT,
    n_batches: int = BPC,
    mid_ht: int = 2,
    th_psum: int = 0,  # 1: tanh -> bf16 PSUM, DVE copies to SBUF (pu_bufs=2)
):
    if th_psum:
        pu_bufs = 2
    nc = bacc.Bacc("TRN2", target_bir_lowering=False, debug=False)

    # Ua weights with the decoder columns prepended (cols 0:8 = dec*16) and
    # padded to 1040 so the DoubleRow pair-dim stride stays 16B-aligned.
    # Fusing dec here avoids an 8-byte-row DMA paying per-descriptor minimums.
    HU = 8 + H + 8  # dec | uaw | pad
    encT = nc.dram_tensor("encT", [BPC, D, T], F8, kind="ExternalInput")
    encN = nc.dram_tensor("encN", [BPC, T, D], BF, kind="ExternalInput")
    uawX = nc.dram_tensor("uawX", [D, HU], F8, kind="ExternalInput")
    wawT = nc.dram_tensor("wawT", [D, H], F8, kind="ExternalInput")
    bsum = nc.dram_tensor("bsum", [1, H], BF, kind="ExternalInput")  # x1024
    vac = nc.dram_tensor("vac", [P, HT], BF, kind="ExternalInput")
    out = nc.dram_tensor("out", [BPC, P, DC], F32, kind="ExternalOutput")

    with tile.TileContext(nc) as tc:
        with (
            tc.tile_pool(name="const", bufs=1) as cpool,
            tc.tile_pool(name="enc", bufs=2) as epool,
            tc.tile_pool(name="work", bufs=2) as wpool,
            tc.tile_pool(name="pu", bufs=pu_bufs, space="PSUM") as pupool,
            tc.tile_pool(name="pc", bufs=2, space="PSUM") as pcpool,
        ):
            # --- resident weights / constants (DMA order matters: the
            # Wa weights come first so the WaPB bias columns are ready
            # when the first tanh fires) ---
            WW = cpool.tile([P, DC, H], F8, tag="WW")
            ww_src = wawT.ap().rearrange("(dc p) h -> p dc h", p=P)
            nc.sync.dma_start(WW[:, :, 0:512], ww_src[:, :, 0:512])
            BS = cpool.tile([1, H], BF, tag="BS")
            nc.sync.dma_start(BS[:], bsum.ap())
            UW = cpool.tile([P, DC, HU], F8, tag="UW")
            uw_src = uawX.ap().rearrange("(dc p) h -> p dc h", p=P)
            nc.sync.dma_start(UW[:, :, 0:520], uw_src[:, :, 0:520])

            ONE8 = cpool.tile([1, BPC], BF, tag="ONE8")
            nc.vector.memset(ONE8[:], 1.0)
            ONEPP = cpool.tile([P, P], BF, tag="ONEPP")
            nc.vector.memset(ONEPP[:], 1.0)

            def enc_dma_t(b):
                EB = epool.tile([P, DC, T], F8, tag="EB", bufs=eb_bufs)
                src = encT.ap()[b].rearrange("(dc p) t -> p dc t", p=P)
                if b == 0:
                    # batch 0 startup: weights arrive in h-column halves
                    # (h-tiles 0-3 need only the first), enc in t-halves —
                    # the first tanh fires ~3us before the weights finish
                    nc.sync.dma_start(EB[:, :, 0:512], src[:, :, 0:512])
                    nc.sync.dma_start(EB[:, :, 512:1024], src[:, :, 512:1024])
                    nc.sync.dma_start(WW[:, :, 512:1024], ww_src[:, :, 512:1024])
                    nc.sync.dma_start(UW[:, :, 520:HU], uw_src[:, :, 520:HU])
                else:
                    nc.sync.dma_start(EB[:], src)
                return EB

            # VAC is only needed by the first score stage; keep it off the
            # startup critical path (WW/UW/EB0 gate the first tanh)
            VAC = cpool.tile([P, HT], BF, tag="VAC")

            def enc_dma_n(b):
                NB = epool.tile([P, TC, D], BF, tag="NB", bufs=nb_bufs)
                nc.sync.dma_start(NB[:], encN.ap()[b].rearrange("(tc p) d -> p tc d", p=P))
                return NB

            # --- WaPB[h, b] = (dec*16) @ (Wa_w*64).T + (Wa_b+Ua_b)*1024,
            # in h-column layout, descaled to fp32 once.  Emitted in halves:
            # h-tiles 4-7 wait for WW's second DMA chunk, so they are spliced
            # into batch 0's Ua stage to keep the PE FIFO unblocked ---
            WaPBc = cpool.tile([P, HT, BPC], F32, tag="WaPBc")

            def prologue(hts):
                for ht in hts:
                    PW = pcpool.tile([P, BPC], F32, tag="sc8", name=f"pw{ht}")
                    for dc in range(DC):
                        nc.tensor.matmul(
                            PW[:],
                            WW[:, dc, ts(ht, P)],
                            UW[:, dc, 0:BPC],
                            start=(dc == 0),
                            stop=False,
                        )
                    nc.tensor.matmul(
                        PW[:], BS[:, ts(ht, P)], ONE8[:], start=False, stop=True
                    )
                    nc.vector.tensor_scalar_mul(WaPBc[:, ht, :], PW[:], 1.0 / TOT_SCALE)

            prologue(range(0, HT // 2))

            def ua_stage(b, EB, mid=None):
                """64 DoubleRow matmuls -> 8 fused bias+tanh tiles [128h, 1024t].

                ``mid`` (the previous batch's score/ctx emission) is spliced in
                after a couple of h-tiles so its exp lands early in the
                strict-FIFO Activation queue without gating this batch's first
                tanh."""
                THs = []
                for ht in range(HT):
                    split0 = b == 0 and ht == 0
                    if split0:
                        # separate per-half PSUM tiles: PSUM deps are
                        # whole-tile, so one [P, T] tile would make the first
                        # half-tanh wait for the second half's matmul group
                        PUh = [
                            pupool.tile([P, 512], F32, tag="pu", name=f"pu_{b}_{ht}_{th}")
                            for th in range(2)
                        ]
                    else:
                        PU = pupool.tile([P, T], F32, tag="pu", name=f"pu_{b}_{ht}")
                    for th in range(2):
                        dst = PUh[th][:] if split0 else PU[:, ts(th, 512)]
                        for g in range(DC // 2):
                            nc.tensor.matmul(
                                dst,
                                UW[:, 2 * g : 2 * g + 2, 8 + ht * P : 8 + (ht + 1) * P],
                                EB[:, 2 * g : 2 * g + 2, ts(th, 512)],
                                start=(g == 0),
                                stop=(g == DC // 2 - 1),
                                perf_mode=DR,
                            )
                    if b == 0 and ht == 2:
                        prologue(range(HT // 2, HT))
                    TH = wpool.tile([P, T], BF, tag="TH", bufs=th_bufs, name=f"th_{b}_{ht}")
                    if th_psum:
                        # tanh lands in a bf16 PSUM tile (cheaper ScalarE
                        # access) and the idle DVE stages it to SBUF for the
                        # score matmuls
                        THp = pcpool.tile([P, T], BF, tag="thp", bufs=2, name=f"thp_{b}_{ht}")
                        act_out = THp
                    else:
                        act_out = TH
                    if split0:
                        # batch 0's first tile activates in t-halves so the
                        # ScalarE stream starts before EB0 fully lands
                        for th in range(2):
                            nc.scalar.activation(
                                act_out[:, ts(th, 512)],
                                PUh[th][:],
                                AF.Tanh,
                                bias=WaPBc[:, ht, b : b + 1],
                                scale=1.0 / TOT_SCALE,
                            )
                    else:
                        nc.scalar.activation(
                            act_out[:],
                            PU[:],
                            AF.Tanh,
                            bias=WaPBc[:, ht, b : b + 1],
                            scale=1.0 / TOT_SCALE,
                        )
                    if th_psum:
                        nc.vector.tensor_copy(TH[:], THp[:])
                    THs.append(TH)
                    if ht == mid_ht - 1 and mid is not None:
                        mid()
                if mid is not None and mid_ht > HT:
                    mid()
                return THs

            def score_ctx_stage(b, THs, NB):
                # scores in column layout [128t', tci]: one accumulation
                # group, 64 K=128/N=1 matmuls with the Va column moving
                SCp = pcpool.tile([P, TC], F32, tag="sc8", name=f"sc{b}")
                # ht-outer: the matmuls gated by the last tanh tile are the
                # final 8 of the group, so exp fires right after it lands
                for ht in range(HT):
                    for tci in range(TC):
                        nc.tensor.matmul(
                            SCp[:, tci : tci + 1],
                            THs[ht][:, ts(tci, P)],
                            VAC[:, ht : ht + 1],
                            start=(tci == 0 and ht == 0),
                            stop=(tci == TC - 1 and ht == HT - 1),
                        )
                EW = wpool.tile([P, TC], BF, tag="EW")
                nc.scalar.activation(EW[:], SCp[:], AF.Exp)
                ES = wpool.tile([P, 1], BF, tag="ES")
                # bf16 partial sums of 8 positive exps: ~0.4% on the softmax
                # denominator, far under the fp8 quantization noise
                with nc.allow_low_precision(reason="softmax denom in bf16"):
                    nc.vector.tensor_reduce(
                        ES[:], EW[:], axis=mybir.AxisListType.X, op=ALU.add
                    )
                # ctx reuses the sc8 PSUM ring: SCp dies at the exp, so the
                # ring alternates SC_b / CTX_b with bufs=2.  Column 8 holds
                # sum(exp) replicated to every partition (all-ones stationary
                # matmul), skipping a gpsimd broadcast hop; its group runs
                # after the ctx group (it only gates the final scale), so the
                # PE never stalls on the DVE reduce before starting ctx.
                CTXp = pcpool.tile([P, 12], F32, tag="sc8", name=f"ctx{b}")
                for dc in range(DC):
                    for tci in range(TC):
                        nc.tensor.matmul(
                            CTXp[:, dc : dc + 1],
                            NB[:, tci, ts(dc, P)],
                            EW[:, tci : tci + 1],
                            start=(dc == 0 and tci == 0),
                            stop=(dc == DC - 1 and tci == TC - 1),
                        )
                nc.tensor.matmul(
                    CTXp[:, 8:9], ONEPP[:], ES[:], start=True, stop=True
                )
                INV128 = wpool.tile([P, 1], F32, tag="INV128")
                nc.vector.reciprocal(INV128[:], CTXp[:, 8:9])
                OUTt = wpool.tile([P, DC], F32, tag="OUTt", bufs=4)
                nc.vector.tensor_scalar_mul(OUTt[:], CTXp[:, 0:DC], INV128[:])
                # mid-stream stores ride the software DGE so they never
                # head-of-line block the input loads on the sync queue; the
                # last one takes the idle Activation HWDGE (lower latency)
                if b == n_batches - 1:
                    nc.scalar.dma_start(out.ap()[b], OUTt[:])
                else:
                    nc.gpsimd.dma_start(out.ap()[b], OUTt[:])

            # --- software pipeline: EB loads run one batch ahead; the
            # score/ctx stage of batch b-1 is emitted BEFORE the Ua stage
            # of b so its exp/ctx work isn't stuck behind b's tanh block
            # in the strict-FIFO Activation queue ---
            EBs = {0: enc_dma_t(0)}
            nc.sync.dma_start(VAC[:], vac.ap())
            prev = None
            for b in range(n_batches):
                if b + 1 < n_batches:
                    EBs[b + 1] = enc_dma_t(b + 1)
                NB = enc_dma_n(b)
                mid = None
                if prev is not None:
                    pb, pTHs, pNB = b - 1, prev[0], prev[1]
                    mid = lambda: score_ctx_stage(pb, pTHs, pNB)
                THs = ua_stage(b, EBs.pop(b), mid=mid)
                prev = (THs, NB)
            score_ctx_stage(n_batches - 1, *prev)

    nc.finalize()
    return nc


_NC = None


def _get_nc():
    global _NC
    if _NC is None:
        _NC = build_bass()
    return _NC


LAST_RESULTS = None


def prepare_in_maps(inputs) -> list:
    enc = np.asarray(inputs["encoder_outputs"], dtype=np.float32)  # [B, T, D]
    dec = np.asarray(inputs["decoder_outputs"], dtype=np.float32)[:, 0, :]  # [B, D]
    Wa_w = np.asarray(inputs["Wa_w"], dtype=np.float32)
    Wa_b = np.asarray(inputs["Wa_b"], dtype=np.float32)
    Ua_w = np.asarray(inputs["Ua_w"], dtype=np.float32)
    Ua_b = np.asarray(inputs["Ua_b"], dtype=np.float32)
    Va_w = np.asarray(inputs["Va_w"], dtype=np.float32)
    # Va_b dropped: softmax(s + c) == softmax(s)

    bf16 = ml_dtypes.bfloat16
    f8 = ml_dtypes.float8_e4m3

    encN_all = enc.astype(bf16).reshape(NCORES, BPC, T, D)
    encT_all = (
        (np.ascontiguousarray(enc.transpose(0, 2, 1)) * ENC_SCALE)
        .astype(f8)
        .reshape(NCORES, BPC, D, T)
    )
    decT_all = (
        np.ascontiguousarray(dec.reshape(NCORES, BPC, D).transpose(0, 2, 1)) * ENC_SCALE
    ).astype(f8)  # [NCORES, D, BPC]
    uawT8 = (np.ascontiguousarray(Ua_w.T) * W_SCALE).astype(f8)
    wawT8 = (np.ascontiguousarray(Wa_w.T) * W_SCALE).astype(f8)
    pad8 = np.zeros((D, 8), dtype=f8)
    uawX_all = [
        np.ascontiguousarray(np.concatenate([decT_all[c], uawT8, pad8], axis=1))
        for c in range(NCORES)
    ]
    bsumS = ((Wa_b + Ua_b) * TOT_SCALE).reshape(1, H).astype(bf16)
    vac = np.ascontiguousarray(Va_w.reshape(HT, P).T).astype(bf16)  # [P, HT]

    return [
        {
            "encT": np.ascontiguousarray(encT_all[c]),
            "encN": np.ascontiguousarray(encN_all[c]),
            "uawX": uawX_all[c],
            "wawT": wawT8,
            "bsum": bsumS,
            "vac": vac,
        }
        for c in range(NCORES)
    ]


def kernel(**inputs) -> np.ndarray:
    in_maps = prepare_in_maps(inputs)
    nc = _get_nc()
    trace = bool(int(os.environ.get("KERNEL_TRACE", "0")))
    try:
        res = run_bass_kernel_spmd(
            nc, in_maps, core_ids=list(range(NCORES)), trace=trace
        )
    except ModuleNotFoundError:
        # axon clients without the NTFF hook (antenv.axon_hooks) cannot trace;
        # retry untraced rather than failing the whole run
        os.environ["BASS_NEVER_TRACE"] = "1"
        res = run_bass_kernel_spmd(
            nc, in_maps, core_ids=list(range(NCORES)), trace=False
        )
    global LAST_RESULTS
    LAST_RESULTS = res

    # out[b, p, dc] holds ctx[b, dc*128 + p]
    outs = [
        res.results[c]["out"].transpose(0, 2, 1).reshape(BPC, D)
        for c in range(NCORES)
    ]
    return np.concatenate(outs, axis=0).reshape(B, 1, D).astype(np.float32)


# revision 45
# speedup vs baseline: 3.4131x; 1.0044x over previous
"""Bahdanau additive attention kernel for 8 Trainium2 NeuronCores.

Data-parallel over batch: B=64 -> 8 batches per core. No collectives.

Per-batch math (reference):
  Wa   = dec @ Wa_w.T + Wa_b                       [1, H]
  Ua   = enc @ Ua_w.T + Ua_b                       [Te, H]
  s    = tanh(Ua + Wa) @ Va_w.T  (+ Va_b, dropped: softmax shift-invariant)
  w    = softmax(s)                                 [Te]
  ctx  = w @ enc                                    [1, De]

Layout: the big Ua matmul runs in the H-on-partitions orientation
(out[h, t]) with the Ua weights stationary and the fp8 encoder tiles
moving (perf_mode=DoubleRow).  That makes the Wa+bias term a
per-partition scalar, so it fuses into the tanh on ScalarE
(out = tanh(psum/1024 + bias)), and Va becomes the moving operand of
K=128 N=1 score-reduction matmuls (scores accumulate directly in score
column layout [128t', tci]).  exp runs on a [128, 8] tile; the context
is again N=1 matmuls with the bf16 enc[t, d] tiles stationary.

fp8 scaling: enc is quantized as enc*16 and the Ua/Wa weights as w*64
(all powers of two, undone exactly by the tanh activation's scale
argument 1/1024).  The scaling moves the small-magnitude weights out of
the fp8e4m3 subnormal range, which is worth ~20% of the quantization
error budget vs naive casting.

Engine budget per batch (cost model): TensorE 64 DoubleRow matmuls at
N=512 ~6.8us, ScalarE 8 tanh ops over [128, 1024] ~8.4us, DMA 1MB fp8
encT + 2MB bf16 encN ~8.7us.  DMA and ScalarE are the pacing resources.
"""

import os
import sys

import numpy as np
import ml_dtypes

for _p in ("/opt/trn_rl_repo",):
    if _p not in sys.path and os.path.isdir(_p):
        sys.path.append(_p)

import concourse.bass as bass
import concourse.tile as tile
import concourse.mybir as mybir
from concourse import bacc
from concourse import bass_isa
from concourse.bass import ts
from concourse.bass_utils import run_bass_kernel_spmd

B, T, D, H = 64, 1024, 1024, 1024
NCORES = 8
BPC = B // NCORES  # batches per core
P = 128
DC = D // P  # 8 contraction chunks
TC = T // P  # 8 t chunks
HT = H // P  # 8 h chunks

ENC_SCALE = 16.0  # fp8 quantization scale for encoder/decoder activations
W_SCALE = 64.0  # fp8 quantization scale for Ua_w / Wa_w
TOT_SCALE = ENC_SCALE * W_SCALE  # combined scale on the matmul PSUM

BF = mybir.dt.bfloat16
F8 = mybir.dt.float8e4
F32 = mybir.dt.float32
AF = mybir.ActivationFunctionType
ALU = mybir.AluOpType
DR = mybir.MatmulPerfMode.DoubleRow


def build_bass(
    eb_bufs: int = 3,
    nb_bufs: int = 3,
    pu_bufs: int = 3,
    th_bufs: int = 14,
    n_batches: int = BPC,
    mid_ht: int = 2,
    th_psum: int = 0,  # 1: tanh -> bf16 PSUM, DVE copies to SBUF (pu_bufs=2)
):
    if th_psum:
        pu_bufs = 2
    nc = bacc.Bacc("TRN2", target_bir_lowering=False, debug=False)

    # Ua weights with the decoder columns prepended (cols 0:8 = dec*16) and
    # padded to 1040 so the DoubleRow pair-dim stride stays 16B-aligned.
    # Fusing dec here avoids an 8-byte-row DMA paying per-descriptor minimums.
    HU = 8 + H + 8  # dec | uaw | pad
    encT = nc.dram_tensor("encT", [BPC, D, T], F8, kind="ExternalInput")
    encN = nc.dram_tensor("encN", [BPC, T, D], BF, kind="ExternalInput")
    uawX = nc.dram_tensor("uawX", [D, HU], F8, kind="ExternalInput")
    wawT = nc.dram_tensor("wawT", [D, H], F8, kind="ExternalInput")
    bsum = nc.dram_tensor("bsum", [1, H], BF, kind="ExternalInput")  # x1024
    vac = nc.dram_tensor("vac", [P, HT], BF, kind="ExternalInput")
    out = nc.dram_tensor("out", [BPC, P, DC], F32, kind="ExternalOutput")

    with tile.TileContext(nc) as tc:
        with (
            tc.tile_pool(name="const", bufs=1) as cpool,
            tc.tile_pool(name="enc", bufs=2) as epool,
            tc.tile_pool(name="work", bufs=2) as wpool,
            tc.tile_pool(name="pu", bufs=pu_bufs, space="PSUM") as pupool,
            tc.tile_pool(name="pc", bufs=2, space="PSUM") as pcpool,
        ):
            # --- resident weights / constants (DMA order matters: the
            # Wa weights come first so the WaPB bias columns are ready
            # when the first tanh fires) ---
            WW = cpool.tile([P, DC, H], F8, tag="WW")
            ww_src = wawT.ap().rearrange("(dc p) h -> p dc h", p=P)
            nc.sync.dma_start(WW[:, :, 0:512], ww_src[:, :, 0:512])
            BS = cpool.tile([1, H], BF, tag="BS")
            nc.sync.dma_start(BS[:], bsum.ap())
            UW = cpool.tile([P, DC, HU], F8, tag="UW")
            uw_src = uawX.ap().rearrange("(dc p) h -> p dc h", p=P)
            nc.sync.dma_start(UW[:, :, 0:520], uw_src[:, :, 0:520])

            ONE8 = cpool.tile([1, BPC], BF, tag="ONE8")
            nc.vector.memset(ONE8[:], 1.0)
            ONEPP = cpool.tile([P, P], BF, tag="ONEPP")
            nc.vector.memset(ONEPP[:], 1.0)

            def enc_dma_t(b):
                EB = epool.tile([P, DC, T], F8, tag="EB", bufs=eb_bufs)
                src = encT.ap()[b].rearrange("(dc p) t -> p dc t", p=P)
                if b == 0:
                    # batch 0 startup: weights arrive in h-column halves
                    # (h-tiles 0-3 need only the first), enc in t-halves —
                    # the first tanh fires ~3us before the weights finish
                    nc.sync.dma_start(EB[:, :, 0:512], src[:, :, 0:512])
                    nc.sync.dma_start(EB[:, :, 512:1024], src[:, :, 512:1024])
                    nc.sync.dma_start(WW[:, :, 512:1024], ww_src[:, :, 512:1024])
                    nc.sync.dma_start(UW[:, :, 520:HU], uw_src[:, :, 520:HU])
                else:
                    nc.sync.dma_start(EB[:], src)
                return EB

            # VAC is only needed by the first score stage; keep it off the
            # startup critical path (WW/UW/EB0 gate the first tanh)
            VAC = cpool.tile([P, HT], BF, tag="VAC")

            def enc_dma_n(b):
                NB = epool.tile([P, TC, D], BF, tag="NB", bufs=nb_bufs)
                nc.sync.dma_start(NB[:], encN.ap()[b].rearrange("(tc p) d -> p tc d", p=P))
                return NB

            # --- WaPB[h, b] = (dec*16) @ (Wa_w*64).T + (Wa_b+Ua_b)*1024,
            # in h-column layout, descaled to fp32 once.  Emitted in halves:
            # h-tiles 4-7 wait for WW's second DMA chunk, so they are spliced
            # into batch 0's Ua stage to keep the PE FIFO unblocked ---
            WaPBc = cpool.tile([P, HT, BPC], F32, tag="WaPBc")

            def prologue(hts):
                for ht in hts:
                    PW = pcpool.tile([P, BPC], F32, tag="sc8", name=f"pw{ht}")
                    for dc in range(DC):
                        nc.tensor.matmul(
                            PW[:],
                            WW[:, dc, ts(ht, P)],
                            UW[:, dc, 0:BPC],
                            start=(dc == 0),
                            stop=False,
                        )
                    nc.tensor.matmul(
                        PW[:], BS[:, ts(ht, P)], ONE8[:], start=False, stop=True
                    )
                    nc.vector.tensor_scalar_mul(WaPBc[:, ht, :], PW[:], 1.0 / TOT_SCALE)

            prologue(range(0, HT // 2))

            def ua_stage(b, EB, mid=None):
                """64 DoubleRow matmuls -> 8 fused bias+tanh tiles [128h, 1024t].

                ``mid`` (the previous batch's score/ctx emission) is spliced in
                after a couple of h-tiles so its exp lands early in the
                strict-FIFO Activation queue without gating this batch's first
                tanh."""
                THs = []
                for ht in range(HT):
                    split0 = b == 0 and ht == 0
                    if split0:
                        # separate per-half PSUM tiles: PSUM deps are
                        # whole-tile, so one [P, T] tile would make the first
                        # half-tanh wait for the second half's matmul group
                        PUh = [
                            pupool.tile([P, 512], F32, tag="pu", name=f"pu_{b}_{ht}_{th}")
                            for th in range(2)
                        ]
                    else:
                        PU = pupool.tile([P, T], F32, tag="pu", name=f"pu_{b}_{ht}")
                    for th in range(2):
                        dst = PUh[th][:] if split0 else PU[:, ts(th, 512)]
                        for g in range(DC // 2):
                            nc.tensor.matmul(
                                dst,
                                UW[:, 2 * g : 2 * g + 2, 8 + ht * P : 8 + (ht + 1) * P],
                                EB[:, 2 * g : 2 * g + 2, ts(th, 512)],
                                start=(g == 0),
                                stop=(g == DC // 2 - 1),
                                perf_mode=DR,
                            )
                    if b == 0 and ht == 2:
                        prologue(range(HT // 2, HT))
                    TH = wpool.tile([P, T], BF, tag="TH", bufs=th_bufs, name=f"th_{b}_{ht}")
                    if th_psum:
                        # tanh lands in a bf16 PSUM tile (cheaper ScalarE
                        # access) and the idle DVE stages it to SBUF for the
                        # score matmuls
                        THp = pcpool.tile([P, T], BF, tag="thp", bufs=2, name=f"thp_{b}_{ht}")
                        act_out = THp
                    else:
                        act_out = TH
                    if split0:
                        # batch 0's first tile activates in t-halves so the
                        # ScalarE stream starts before EB0 fully lands
                        for th in range(2):
                            nc.scalar.activation(
                                act_out[:, ts(th, 512)],
                                PUh[th][:],
                                AF.Tanh,
                                bias=WaPBc[:, ht, b : b + 1],
                                scale=1.0 / TOT_SCALE,
                            )
                    else:
                        nc.scalar.activation(
                            act_out[:],
                            PU[:],
                            AF.Tanh,
                            bias=WaPBc[:, ht, b : b + 1],
                            scale=1.0 / TOT_SCALE,
                        )
                    if th_psum:
                        nc.vector.tensor_copy(TH[:], THp[:])
                    THs.append(TH)
                    if ht == mid_ht - 1 and mid is not None:
                        mid()
                if mid is not None and mid_ht > HT:
                    mid()
                return THs

            def score_ctx_stage(b, THs, NB):
                # scores in column layout [128t', tci]: one accumulation
                # group, 64 K=128/N=1 matmuls with the Va column moving
                SCp = pcpool.tile([P, TC], F32, tag="sc8", name=f"sc{b}")
                # ht-outer: the matmuls gated by the last tanh tile are the
                # final 8 of the group, so exp fires right after it lands
                for ht in range(HT):
                    for tci in range(TC):
                        nc.tensor.matmul(
                            SCp[:, tci : tci + 1],
                            THs[ht][:, ts(tci, P)],
                            VAC[:, ht : ht + 1],
                            start=(tci == 0 and ht == 0),
                            stop=(tci == TC - 1 and ht == HT - 1),
                        )
                EW = wpool.tile([P, TC], BF, tag="EW")
                nc.scalar.activation(EW[:], SCp[:], AF.Exp)
                # ctx reuses the sc8 PSUM ring: SCp dies at the exp, so the
                # ring alternates SC_b / CTX_b with bufs=2.  One accumulation
                # group: column 8 first accumulates sum(exp) replicated to
                # every partition (all-ones stationary matmuls, one per EW
                # column) so the reciprocal fires as early as possible, then
                # columns 0-7 take the context.
                CTXp = pcpool.tile([P, 12], F32, tag="sc8", name=f"ctx{b}")
                # one accumulation group: sum(exp) lands replicated in column
                # 8 (all-ones stationary), context in columns 0-7.  For the
                # last batch the sum goes first so the reciprocal (and with it
                # the final scale+store chain) fires as early as possible;
                # mid-stream the context goes first so the next batch's
                # weights aren't kept waiting.
                ops = [
                    (CTXp[:, 8:9], ONEPP[:], EW[:, tci : tci + 1])
                    for tci in range(TC)
                ]
                ctx_ops = [
                    (CTXp[:, dc : dc + 1], NB[:, tci, ts(dc, P)], EW[:, tci : tci + 1])
                    for dc in range(DC)
                    for tci in range(TC)
                ]
                if b == n_batches - 1:
                    ops = ops + ctx_ops
                else:
                    ops = ctx_ops + ops
                for i, (o, l, r) in enumerate(ops):
                    nc.tensor.matmul(
                        o, l, r, start=(i == 0), stop=(i == len(ops) - 1)
                    )
                INV128 = wpool.tile([P, 1], F32, tag="INV128")
                nc.vector.reciprocal(INV128[:], CTXp[:, 8:9])
                OUTt = wpool.tile([P, DC], F32, tag="OUTt", bufs=4)
                nc.vector.tensor_scalar_mul(OUTt[:], CTXp[:, 0:DC], INV128[:])
                # mid-stream stores ride the software DGE so they never
                # head-of-line block the input loads on the sync queue; the
                # last one takes the idle Activation HWDGE (lower latency)
                if b == n_batches - 1:
                    nc.scalar.dma_start(out.ap()[b], OUTt[:])
                else:
                    nc.gpsimd.dma_start(out.ap()[b], OUTt[:])

            # --- software pipeline: EB loads run one batch ahead; the
            # score/ctx stage of batch b-1 is emitted BEFORE the Ua stage
            # of b so its exp/ctx work isn't stuck behind b's tanh block
            # in the strict-FIFO Activation queue ---
            EBs = {0: enc_dma_t(0)}
            nc.sync.dma_start(VAC[:], vac.ap())
            prev = None
            for b in range(n_batches):
                if b + 1 < n_batches:
                    EBs[b + 1] = enc_dma_t(b + 1)
                NB = enc_dma_n(b)
                mid = None
                if prev is not None:
                    pb, pTHs, pNB = b - 1, prev[0], prev[1]
                    mid = lambda: score_ctx_stage(pb, pTHs, pNB)
                THs = ua_stage(b, EBs.pop(b), mid=mid)
                prev = (THs, NB)
            score_ctx_stage(n_batches - 1, *prev)

    nc.finalize()
    return nc


_NC = None


def _get_nc():
    global _NC
    if _NC is None:
        _NC = build_bass()
    return _NC


LAST_RESULTS = None


def prepare_in_maps(inputs) -> list:
    enc = np.asarray(inputs["encoder_outputs"], dtype=np.float32)  # [B, T, D]
    dec = np.asarray(inputs["decoder_outputs"], dtype=np.float32)[:, 0, :]  # [B, D]
    Wa_w = np.asarray(inputs["Wa_w"], dtype=np.float32)
    Wa_b = np.asarray(inputs["Wa_b"], dtype=np.float32)
    Ua_w = np.asarray(inputs["Ua_w"], dtype=np.float32)
    Ua_b = np.asarray(inputs["Ua_b"], dtype=np.float32)
    Va_w = np.asarray(inputs["Va_w"], dtype=np.float32)
    # Va_b dropped: softmax(s + c) == softmax(s)

    bf16 = ml_dtypes.bfloat16
    f8 = ml_dtypes.float8_e4m3

    encN_all = enc.astype(bf16).reshape(NCORES, BPC, T, D)
    encT_all = (
        (np.ascontiguousarray(enc.transpose(0, 2, 1)) * ENC_SCALE)
        .astype(f8)
        .reshape(NCORES, BPC, D, T)
    )
    decT_all = (
        np.ascontiguousarray(dec.reshape(NCORES, BPC, D).transpose(0, 2, 1)) * ENC_SCALE
    ).astype(f8)  # [NCORES, D, BPC]
    uawT8 = (np.ascontiguousarray(Ua_w.T) * W_SCALE).astype(f8)
    wawT8 = (np.ascontiguousarray(Wa_w.T) * W_SCALE).astype(f8)
    pad8 = np.zeros((D, 8), dtype=f8)
    uawX_all = [
        np.ascontiguousarray(np.concatenate([decT_all[c], uawT8, pad8], axis=1))
        for c in range(NCORES)
    ]
    bsumS = ((Wa_b + Ua_b) * TOT_SCALE).reshape(1, H).astype(bf16)
    vac = np.ascontiguousarray(Va_w.reshape(HT, P).T).astype(bf16)  # [P, HT]

    return [
        {
            "encT": np.ascontiguousarray(encT_all[c]),
            "encN": np.ascontiguousarray(encN_all[c]),
            "uawX": uawX_all[c],
            "wawT": wawT8,
            "bsum": bsumS,
            "vac": vac,
        }
        for c in range(NCORES)
    ]


def kernel(**inputs) -> np.ndarray:
    in_maps = prepare_in_maps(inputs)
    nc = _get_nc()
    trace = bool(int(os.environ.get("KERNEL_TRACE", "0")))
    try:
        res = run_bass_kernel_spmd(
            nc, in_maps, core_ids=list(range(NCORES)), trace=trace
        )
    except ModuleNotFoundError:
        # axon clients without the NTFF hook (antenv.axon_hooks) cannot trace;
        # retry untraced rather than failing the whole run
        os.environ["BASS_NEVER_TRACE"] = "1"
        res = run_bass_kernel_spmd(
            nc, in_maps, core_ids=list(range(NCORES)), trace=False
        )
    global LAST_RESULTS
    LAST_RESULTS = res

    # out[b, p, dc] holds ctx[b, dc*128 + p]
    outs = [
        res.results[c]["out"].transpose(0, 2, 1).reshape(BPC, D)
        for c in range(NCORES)
    ]
    return np.concatenate(outs, axis=0).reshape(B, 1, D).astype(np.float32)
